# revision 1
# baseline (speedup 1.0000x reference)
"""Distributed 2^22-point radix-2 FFT-with-abs-at-every-stage on 8 NeuronCores.

Math: the reference applies abs() after every butterfly stage, so all state is
real and non-negative.  We propagate SQUARED magnitudes v = |y|^2:
    stage s:  q = v_e + v_o ;  r = sqrt(v_e * v_o)
              v_e' = q + 2*cos(2*pi*k/2^s)*r ;  v_o' = q - 2*c*r
(no sines needed: |e + (c - i s) o|^2 = e^2 + o^2 + 2 c e o for e,o >= 0).
o' >= 0 is guaranteed (no relu) because p = v_e*v_o is computed with an fp32
output (exact for fp16 inputs) and the 2c table is shrunk by (1 - 2^-9).

Precision plan (validated numerically against the reference):
  - stages 1..16 run in fp16: every tensor_tensor op gets the DVE 2x_1p
    perf mode (2-byte packed operands), halving DVE time;
  - stages 17..22 run in fp32: the near-DC "hot" bins accumulate relative
    error linearly per stage and dominate the L2 norm, and the late stages
    contribute the most, so they get full precision;
  - v grows ~4^s in the near-DC bins; flat power-of-2 rescales (x 2^-4
    after stages 6, 8, 10, 12, 14) keep fp16 in range.  The product of all
    rescales (2^-20) is undone in the final sqrt's free ACT scale.

Distribution (one all-to-all; everything else core-local; all butterflies
pair along the free dimension so every op uses all 128 partitions):
  - host bit-reverses x and shards contiguously: core d owns bits 21..19 = d
  - layout1 [128, 4096]: partition w = permuted bits 18..12
    (w = [b16 b17 b18 | b12 b13 b14 b15], LSB first), free f = bits 11..0
    -> stages 1..12 pair free bits 0..11
  - PE-transpose of 32 [128,128] blocks + AllToAll resharding bits 11..9
    -> layout2 [128, 4096]: partition ww = bits 6..0,
       f' = [b16 b17 b18 | b12..b15 | b7 b8 | b19 b20 b21]  (LSB first)
    -> stages 13..16 pair f' bits 3..6 (runs >= 8: fp16 2x ok),
       stages 17..19 pair f' bits 0..2 (fp32: packing irrelevant),
       stages 20..22 pair f' bits 9..11.
Engines per stage: DVE: q, t2, e' (+part of o'); Pool (stt form, 0.6 eff):
p (fp32 out), rest of o'; ACT: r = sqrt(p).
"""

import numpy as np

import concourse.bacc as bacc
import concourse.mybir as mybir
import concourse.tile as tile
from concourse.bass_types import AP
from concourse.bass_utils import run_bass_kernel_spmd

FP32 = mybir.dt.float32
FP16 = mybir.dt.float16
AF = mybir.ActivationFunctionType
OP = mybir.AluOpType

NBITS = 22
N = 1 << NBITS
M = 8          # cores
P = 128
F = 4096
FH = F // 2

F16_LAST = 16                      # last fp16 stage (17.. run fp32)
RESCALE_AFTER = (6, 8, 10, 12, 14)  # flat x 2^-4 after these stages
RS = 2.0 ** -4
FINAL_SCALE = float(2.0 ** (4 * len(RESCALE_AFTER)))  # undo in final sqrt
# fp16 twiddles are exact (a shrink factor would bias the near-DC chains
# coherently over 16 stages); tiny negatives from o' = q - t2 cancellation
# are clamped by a cheap per-stage tensor_scalar max(.,0) on the narrow
# |cos|~1 column band.  fp32 stages instead shrink the table by 2^-14
# (bias 6 stages x 2^-15 ~ 2e-4, negligible), which provably keeps o' >= 0.
SHRINK16 = 1.0
SHRINK32 = 1.0 - 2.0 ** -14   # covers ACT sqrt-table relative error

# f'-bit (layout2 free bit) paired by stage s
F_BIT = {}
for _s in range(13, 17):
    F_BIT[_s] = 3 + (_s - 13)      # global bits 12..15
for _s in range(17, 20):
    F_BIT[_s] = _s - 17            # global bits 16..18
for _s in range(20, 23):
    F_BIT[_s] = 9 + (_s - 20)      # global bits 19..21

TWL_OFF = {s: 2048 * (s - 13) for s in range(13, 17)}   # fp16 late table
TWL32_OFF = {s: 2048 * (s - 17) for s in range(17, 23)}  # fp32 late table


def _bitrev_perm():
    x = np.arange(N)
    r = np.zeros(N, dtype=np.int64)
    for b in range(NBITS):
        r = (r << 1) | ((x >> b) & 1)
    return r


def _fprime_to_n(fp, ww, d):
    """global index n from layout2 (core d, partition ww, free col f')."""
    return (ww
            + ((fp >> 0) & 7) * (1 << 16)
            + ((fp >> 3) & 15) * (1 << 12)
            + ((fp >> 7) & 3) * (1 << 7)
            + d * (1 << 9)
            + ((fp >> 9) & 7) * (1 << 19))


# --------------------------------------------------------------------------
# host-side tables / permutations (cached)
# --------------------------------------------------------------------------
_HOST_CACHE = {}


def _in_perm():
    """perm[d, w, f] -> index into x (full input)."""
    if "in" not in _HOST_CACHE:
        br = _bitrev_perm()
        w = np.arange(P)[:, None]
        f = np.arange(F)[None, :]
        l = f + ((w >> 3) & 15) * (1 << 12) + (w & 7) * (1 << 16)
        out = np.empty((M, P, F), dtype=np.int64)
        for d in range(M):
            out[d] = br[d * (1 << 19) + l]
        _HOST_CACHE["in"] = out
    return _HOST_CACHE["in"]


def _out_perm():
    """perm[d, ww, f'] -> global output index n."""
    if "out" not in _HOST_CACHE:
        ww = np.arange(P)[:, None]
        fp = np.arange(F)[None, :]
        out = np.empty((M, P, F), dtype=np.int64)
        for d in range(M):
            out[d] = _fprime_to_n(fp, ww, d)
        _HOST_CACHE["out"] = out
    return _HOST_CACHE["out"]


def host_tw_early():
    """[P, 4094] fp16: stages 2..12, cols 2^(s-1) each, value 2c*(1-2^-9),
    identical rows (twiddle depends only on free bits there)."""
    if "twE" in _HOST_CACHE:
        return _HOST_CACHE["twE"]
    cols = []
    for s in range(2, 13):
        h = 1 << (s - 1)
        t = np.arange(h)
        cols.append(2.0 * np.cos(2 * np.pi * t / (1 << s)) * SHRINK16)
    row = np.concatenate(cols).astype(np.float16)
    _HOST_CACHE["twE"] = np.broadcast_to(row, (P, row.size)).copy()
    return _HOST_CACHE["twE"]


TWE_OFF = {}
_c = 0
for _s in range(2, 13):
    TWE_OFF[_s] = _c
    _c += 1 << (_s - 1)
TWE_COLS = _c


def _late_table(s, d, dtype, shrink):
    """[P, 2048] twiddle table for stage s, core d: col idx = f' with
    pairing bit F_BIT[s] removed; value 2*cos(2*pi*k/2^s)*shrink."""
    b = F_BIT[s]
    idx = np.arange(2048)
    fp = (idx & ((1 << b) - 1)) | ((idx >> b) << (b + 1))
    ww = np.arange(P)[:, None]
    n = _fprime_to_n(fp[None, :], ww, d)
    k = n % (1 << (s - 1))
    return (2.0 * np.cos(2 * np.pi * k / (1 << s)) * shrink).astype(dtype)


def host_tw_late16(d):
    key = ("twL16", d)
    if key not in _HOST_CACHE:
        _HOST_CACHE[key] = np.concatenate(
            [_late_table(s, d, np.float16, SHRINK16) for s in range(13, 17)],
            axis=1)
    return _HOST_CACHE[key]


def host_tw_late32(d):
    key = ("twL32", d)
    if key not in _HOST_CACHE:
        _HOST_CACHE[key] = np.concatenate(
            [_late_table(s, d, np.float32, SHRINK32) for s in range(17, 23)],
            axis=1)
    return _HOST_CACHE[key]


# --------------------------------------------------------------------------
# device program
# --------------------------------------------------------------------------
def _pair_views(v_ap, b):
    """(e, o) views of [P, 4096] v pairing free bit b: dims
    [p, [2^(b+1), 2^(11-b)], [1, 2^b]]."""
    vv = v_ap.rearrange("p (o pair i) -> p o pair i", pair=2, i=1 << b)
    return vv[:, :, 0, :], vv[:, :, 1, :]


def _packed(ap2048, b):
    """[P, 2048] scratch/table AP shaped to match _pair_views dims."""
    if b == 0:
        return ap2048.rearrange("p (o i) -> p o i", i=1)
    return ap2048.rearrange("p (o i) -> p o i", i=1 << b)


def build_nc(stop_after=None, no_cc=False, NCH=4, O16=0.875, O32=0.25):
    nc = bacc.Bacc()

    x_in = nc.dram_tensor("x", [P, F], FP32, kind="ExternalInput")
    twe_in = nc.dram_tensor("twe", [P, TWE_COLS], FP16, kind="ExternalInput")
    twl16_in = nc.dram_tensor("twl16", [P, 8192], FP16, kind="ExternalInput")
    twl32_in = nc.dram_tensor("twl32", [P, 12288], FP32, kind="ExternalInput")
    ident_in = nc.dram_tensor("ident", [P, P], FP16, kind="ExternalInput")
    out = nc.dram_tensor("out", [P, F], FP32, kind="ExternalOutput")

    with tile.TileContext(nc) as tc:
        _handles = []
        sendh = []
        recvh = []
        for uh in range(4):
            s_t, _h1 = tc.tile([M, P, 128], FP16, space="DRAM",
                               name=f"a2a_send{uh}")
            r_t, _h2 = tc.tile([M, P, 128], FP16, space="DRAM",
                               addr_space="Shared", name=f"a2a_recv{uh}")
            _handles += [_h1, _h2]
            sendh.append(s_t)
            recvh.append(r_t)

        v16, _h3 = tc.tile([P, F], FP16, name="v16")     # layout1 state
        v2, _h4 = tc.tile([P, F], FP16, name="v2")       # layout2 fp16 state
        v32, _h5 = tc.tile([P, F], FP32, name="v32")     # layout2 fp32 state
        twe, _h6 = tc.tile([P, TWE_COLS], FP16, name="twe")
        twl16, _h7 = tc.tile([P, 8192], FP16, name="twl16")
        twl32, _h8 = tc.tile([P, 12288], FP32, name="twl32")
        ident_t, _h9 = tc.tile([P, P], FP16, name="ident")

        with (
            tc.tile_pool(name="io", bufs=1) as io_pool,
            tc.tile_pool(name="scratch", bufs=4) as sp,
            tc.tile_pool(name="psum", bufs=4, space="PSUM") as pp,
        ):
            xt = io_pool.tile([P, F], FP32, tag="io")
            for ci in range(4):
                nc.sync.dma_start(xt[:, ci * 1024:(ci + 1) * 1024],
                                  x_in[:, ci * 1024:(ci + 1) * 1024])
            # twe is needed by stage 2 (~8us): one early trigger on the ACT
            # queue (dispatch cost ~0.7us, before any ACT compute is ready).
            nc.scalar.dma_start(twe[:], twe_in[:])
            # Everything else rides the otherwise-idle SP queue behind x:
            # twl16 needed at ~55us, twl32 at ~100us; SP finishes by ~35us.
            nc.sync.dma_start(ident_t[:], ident_in[:])
            for ci in range(2):
                nc.sync.dma_start(twl16[:, ci * 4096:(ci + 1) * 4096],
                                  twl16_in[:, ci * 4096:(ci + 1) * 4096])
            for ci in range(3):
                nc.sync.dma_start(twl32[:, ci * 4096:(ci + 1) * 4096],
                                  twl32_in[:, ci * 4096:(ci + 1) * 4096])

            def bail(label, ap):
                if stop_after == label:
                    ot = io_pool.tile([P, F], FP32, tag="bailio")
                    nc.scalar.copy(ot[:], ap)
                    nc.sync.dma_start(out[:], ot[:])
                    return True
                return False

            # ---------------- stage 1 (fp32 in -> fp16) ----------------
            # chunks align 1:1 with the x DMA chunks for early start
            xe = xt[:].rearrange("p (c pair) -> p c pair", pair=2)[:, :, 0]
            xo = xt[:].rearrange("p (c pair) -> p c pair", pair=2)[:, :, 1]
            ve = v16[:].rearrange("p (c pair) -> p c pair", pair=2)[:, :, 0]
            vo = v16[:].rearrange("p (c pair) -> p c pair", pair=2)[:, :, 1]
            s1 = []
            for c in range(4):
                sl = slice(c * 512, (c + 1) * 512)
                st = sp.tile([P, 512], FP16, tag="q")
                dt = sp.tile([P, 512], FP32, tag="t")
                nc.vector.tensor_tensor(st[:], xe[:, sl], xo[:, sl], OP.add)
                nc.gpsimd.tensor_tensor(dt[:], xe[:, sl], xo[:, sl],
                                        OP.subtract)
                s1.append((sl, st, dt))
            for (sl, st, dt) in s1:
                nc.scalar.activation(ve[:, sl], st[:], AF.Square)
                nc.scalar.activation(vo[:, sl], dt[:], AF.Square)
            stopped = bail('stage1', v16[:])

            # ---------------- generic stage ----------------
            def do_stage(s, v_ap, tw_tile, tw_off, fp16, o_dve_frac,
                         vout_ap=None, nch=None):
                """One butterfly stage, vin -> vout ([P, 4096]); b = paired
                free bit; chunked along the widest dim, ops emitted
                phase-by-phase across chunks for engine pipelining."""
                if vout_ap is None:
                    vout_ap = v_ap
                if nch is None:
                    nch = NCH
                b = (s - 1) if s <= 12 else F_BIT[s]
                e_all, o_all = _pair_views(v_ap, b)
                eo_all, oo_all = _pair_views(vout_ap, b)
                n_out = 1 << (11 - b)   # outer count
                inner = 1 << b
                dt_state = FP16 if fp16 else FP32
                # chunk grid: outer split first, inner split for the rest
                no = min(nch, n_out)
                ni = max(1, nch // no)
                ni = min(ni, max(1, inner // 8))
                nblk = n_out // no
                inner_c = inner // ni
                ccols = nblk * inner_c
                chunks = [(slice(c * nblk, (c + 1) * nblk),
                           slice(i * inner_c, (i + 1) * inner_c))
                          for c in range(no) for i in range(ni)]
                cd = []
                for (co, cin) in chunks:
                    e = e_all[:, co, cin]
                    o = o_all[:, co, cin]
                    eo = eo_all[:, co, cin]
                    oo = oo_all[:, co, cin]
                    if s <= 12:
                        t0 = TWE_OFF[s]
                        tw = AP(twe.tensor,
                                twe[:, t0 + (cin.start or 0):].offset,
                                [twe[:].ap[0], [0, nblk], [1, inner_c]])
                    else:
                        t0 = tw_off[s]
                        twp = _packed(tw_tile[:, t0:t0 + 2048], b)
                        tw = twp[:, co, cin]
                    q = sp.tile([P, ccols], dt_state, tag="q")
                    p32 = sp.tile([P, ccols], FP32, tag="p")
                    r2 = sp.tile([P, ccols], dt_state, tag="r")
                    t2 = sp.tile([P, ccols], dt_state, tag="t")
                    mk = lambda t_: t_.rearrange("p (o i) -> p o i",
                                                 i=inner_c)
                    cd.append(dict(e=e, o=o, eo=eo, oo=oo, tw=tw, q=q,
                                   p32=p32, r2=r2, t2=t2, qv=mk(q[:]),
                                   pv=mk(p32[:]), rv=mk(r2[:]),
                                   tv=mk(t2[:])))
                for c in cd:
                    nc.vector.tensor_tensor(c["qv"], c["e"], c["o"], OP.add)
                    nc.gpsimd.tensor_tensor(c["pv"], c["e"], c["o"], OP.mult)
                for c in cd:
                    nc.scalar.activation(c["r2"][:], c["p32"][:], AF.Sqrt)
                for ci, c in enumerate(cd):
                    nc.vector.tensor_tensor(c["tv"], c["tw"], c["rv"],
                                            OP.mult)
                    nc.vector.tensor_tensor(c["eo"], c["qv"], c["tv"],
                                            OP.add)
                    # o' split DVE / Pool: prefer the last dim (keeps the
                    # split possible even when the chunk has one block)
                    oo, qv, tv = c["oo"], c["qv"], c["tv"]
                    if inner_c >= 16:
                        k = min(inner_c - 8,
                                max(8, int(inner_c * o_dve_frac)) & ~7)
                        nc.vector.tensor_tensor(oo[:, :, 0:k], qv[:, :, 0:k],
                                                tv[:, :, 0:k], OP.subtract)
                        nc.gpsimd.tensor_tensor(
                            oo[:, :, k:inner_c], qv[:, :, k:inner_c],
                            tv[:, :, k:inner_c], OP.subtract)
                    else:
                        k = max(1, min(nblk - 1, int(nblk * o_dve_frac))) \
                            if nblk > 1 else nblk
                        nc.vector.tensor_tensor(oo[:, 0:k, :], qv[:, 0:k, :],
                                                tv[:, 0:k, :], OP.subtract)
                        if k < nblk:
                            nc.gpsimd.tensor_tensor(
                                oo[:, k:nblk, :], qv[:, k:nblk, :],
                                tv[:, k:nblk, :], OP.subtract)

            # ---------------- stages 2..12 (fp16, layout1) ----------------
            def rescale(v_ap):
                # chunked so it isn't a whole-tile barrier
                for c in range(4):
                    sl = slice(c * 1024, (c + 1) * 1024)
                    nc.vector.tensor_scalar(v_ap[:, sl], v_ap[:, sl], RS,
                                            None, OP.mult)

            def clamp_band(v_ap, s):
                """max(v, 0) on the cancellation band: the o'-side of k~0
                and the e'-side of k~h pairs, one contiguous range [h-g,
                h+g) per 2h-block."""
                h = 1 << (s - 1)
                m = 2 * h
                g = max(1, h // 16)
                base = v_ap[:, h - g:]
                band = AP(base.tensor, base.offset,
                          [base.ap[0], [m, F // m], [1, 2 * g]])
                nc.vector.tensor_scalar(band, band, 0.0, None, OP.max)

            for s in range(2, 13):
                if stopped:
                    break
                do_stage(s, v16[:], None, None, True, O16)
                clamp_band(v16[:], s)
                if s in RESCALE_AFTER and s != 12:
                    rescale(v16[:])
                stopped = stopped or bail(f'stage{s}', v16[:])

            # ------------- transpose + all-to-all (4 u-quarters) ----------
            # stage-12 rescale is folded into the PSUM->SBUF copies
            stg = io_pool.tile([P, F], FP16, tag="io2")
            for u in range(4):
                if stopped:
                    break
                for bi in range(0, M, 4):
                    pt = pp.tile([P, 512], FP16, tag="pt")
                    for j, dp in enumerate(range(bi, bi + 4)):
                        c0 = dp * 512 + u * 128
                        nc.tensor.transpose(pt[:, j * 128:(j + 1) * 128],
                                            v16[:, c0:c0 + 128], ident_t[:])
                    # copies spread across engines, rescale folded in
                    for j, dp in enumerate(range(bi, bi + 4)):
                        dst = stg[:, dp * 512 + u * 128:
                                  dp * 512 + u * 128 + 128]
                        src = pt[:, j * 128:(j + 1) * 128]
                        eng = (u + bi // 4 + j) % 2
                        if eng == 0:
                            nc.scalar.mul(dst, src, RS)
                        else:
                            nc.vector.tensor_scalar(dst, src, RS, None,
                                                    OP.mult)
                # sends + recvs on SP, transport on ACT: the phases of the
                # four quarters pipeline across the two queues
                nc.sync.dma_start(
                    sendh[u][:].rearrange("d w c -> w d c"),
                    stg[:].rearrange("w (d u c) -> w d u c",
                                     d=M, u=4)[:, :, u, :])
                if no_cc:
                    nc.scalar.dma_start(recvh[u][:], sendh[u][:])
                else:
                    nc.gpsimd.collective_compute(
                        "AllToAll", OP.bypass,
                        replica_groups=[list(range(M))],
                        ins=[sendh[u][:].opt()],
                        outs=[recvh[u][:].opt()])
                nc.sync.dma_start(
                    v2[:].rearrange("w (s u c) -> w s u c",
                                    s=M, u=4)[:, :, u, :],
                    recvh[u][:].rearrange("s w c -> w s c"))
            stopped = stopped or bail('a2a', v2[:])

            # ------------- stages 13..16 (fp16, layout2) -------------
            # cancellation bands (per stage: [(col-offset, free-dims)]):
            # k ~ 0 -> o'-side (pairing bit set), low k-bits zero; k ~
            # 2^(s-1) -> e'-side mirror.  Over-clamping safe cols is
            # harmless (max(v,0) only changes rounding-level negatives).
            CLAMP2 = {
                13: [(8, [[512, 8], [16, 8], [1, 8]]),
                     (384, [[512, 8], [16, 8], [1, 8]])],
                14: [(16, [[512, 8], [32, 4], [1, 8]]),
                     (392, [[512, 8], [32, 4], [1, 8]])],
                15: [(32, [[512, 8], [64, 4], [1, 8]]),
                     (280, [[512, 8], [64, 4], [1, 8]])],
                16: [(64, [[128, 32], [1, 8]]),
                     (56, [[128, 32], [1, 8]])],
            }

            def clamp_band2(v_ap, s):
                for off, dims in CLAMP2[s]:
                    base = v_ap[:, off:]
                    band = AP(base.tensor, base.offset,
                              [base.ap[0]] + dims)
                    nc.vector.tensor_scalar(band, band, 0.0, None, OP.max)

            for s in range(13, 17):
                if stopped:
                    break
                do_stage(s, v2[:], twl16, TWL_OFF, True, O16)
                clamp_band2(v2[:], s)
                if s in RESCALE_AFTER:
                    rescale(v2[:])
                stopped = stopped or bail(f'stage{s}', v2[:])

            # ------------- stage 17 (fp16 -> fp32) -------------
            if not stopped:
                do_stage(17, v2[:], twl32, TWL32_OFF, False, O32,
                         vout_ap=v32[:])
                stopped = bail('stage17', v32[:])

            # ------------- stages 18..22 (fp32, layout2) -------------
            for s in range(18, 23):
                if stopped:
                    break
                do_stage(s, v32[:], twl32, TWL32_OFF, False, O32)
                stopped = stopped or bail(f'stage{s}', v32[:])

            # ---------------- final magnitudes ----------------
            if not stopped:
                ot = io_pool.tile([P, F], FP32, tag="io")
                for c in range(8):
                    sl = slice(c * 512, (c + 1) * 512)
                    nc.scalar.activation(ot[:, sl], v32[:, sl], AF.Sqrt,
                                         scale=FINAL_SCALE)
                    if c % 2 == 0:
                        nc.sync.dma_start(out[:, sl], ot[:, sl])
                    else:
                        nc.scalar.dma_start(out[:, sl], ot[:, sl])

    nc.finalize()
    return nc


_NC_CACHE = None


def _get_nc():
    global _NC_CACHE
    if _NC_CACHE is None:
        _NC_CACHE = build_nc()
    return _NC_CACHE


def host_inputs(x):
    perm = _in_perm()
    xv = x[perm]          # [M, P, F] fp32
    ident = np.eye(P, dtype=np.float16)
    twe = host_tw_early()
    return [dict(x=xv[d], twe=twe, twl16=host_tw_late16(d),
                 twl32=host_tw_late32(d), ident=ident) for d in range(M)]


def assemble(outs):
    operm = _out_perm()
    full = np.empty(N, dtype=np.float32)
    for d in range(M):
        full[operm[d].reshape(-1)] = np.asarray(outs[d]).reshape(-1)
    return full


def kernel(x: np.ndarray) -> np.ndarray:
    x = np.asarray(x)
    assert x.shape == (N,) and x.dtype == np.float32, (x.shape, x.dtype)
    in_maps = host_inputs(x)
    nc = _get_nc()
    res = run_bass_kernel_spmd(nc, in_maps, core_ids=list(range(M)))
    return assemble([res.results[d]["out"] for d in range(M)])


if __name__ == "__main__":
    rng = np.random.default_rng(0)
    x = rng.standard_normal(N).astype(np.float32)
    r = kernel(x)
    print("kernel ran, out[:4] =", r[:4])



# revision 15
# speedup vs baseline: 1.1066x; 1.1066x over previous
"""Distributed 2^22-point radix-2 FFT-with-abs-at-every-stage on 8 NeuronCores.

Math: the reference applies abs() after every butterfly stage, so all state is
real and non-negative.  We propagate SQUARED magnitudes v = |y|^2:
    stage s:  q = v_e + v_o ;  r = sqrt(v_e * v_o)
              v_e' = q + 2*cos(2*pi*k/2^s)*r ;  v_o' = q - 2*c*r
(no sines needed: |e + (c - i s) o|^2 = e^2 + o^2 + 2 c e o for e,o >= 0).

Precision: stages 1..16 fp16 (DVE 2x_1p packing), 17..22 fp32.  Flat x2^-4
rescales after stages 6,8,10,12,14 keep fp16 in range; the product (2^-20)
is undone in the final sqrt's free ACT scale.  Rescales fold into the stage
(q scaled by one TS op, r scaled inside the ACT sqrt) - never a separate
pass.  Input is cast fp16 on host; output is fp16 scaled by 2^-7 and upcast
on host (power-of-2 scales are error-free).

Distribution (one all-to-all; butterflies pair along the free dimension):
  - host bit-reverses x and shards contiguously: core d owns bits 21..19 = d
  - layout1 [128, 4096]: partition w = permuted bits 18..12
    (w = [b16 b17 b18 | b12 b13 b14 b15], LSB first), free f = bits 11..0
    -> stages 1..12 pair free bits 0..11
  - PE-transpose + AllToAll resharding -> layout2 [128, 4096]:
    partition ww = bits 6..0, f' = [b16 b17 b18 | b12..b15 | b7 b8 | b19..b21]
    -> stages 13..16 pair f' bits 3..6 (fp16), 17..19 pair f' 0..2 (fp32),
       20..22 pair f' 9..11 (fp32).
Pipelining: f bits 7..8 (= f' bits 7..8 = the a2a quarter index u) are last
paired at stage 9, so stages 10..22 + transpose + a2a + final output all run
per-quarter: a 4-deep pipeline that hides the collective and output DMA.
Engines: DVE: q, t2, e', most of o'; Pool: p = v_e*v_o (fp32 out), rest of
o'; ACT: r = sqrt(p) (+ folded rescale), PSUM drains, final sqrt.
"""

import numpy as np

import concourse.bacc as bacc
import concourse.mybir as mybir
import concourse.tile as tile
from concourse.bass_types import AP
from concourse.bass_utils import run_bass_kernel_spmd

FP32 = mybir.dt.float32
FP16 = mybir.dt.float16
AF = mybir.ActivationFunctionType
OP = mybir.AluOpType

NBITS = 22
N = 1 << NBITS
M = 8          # cores
P = 128
F = 4096

RESCALE_AFTER = (6, 8, 10, 12, 14)
RS = 2.0 ** -4
FINAL_SCALE = float(2.0 ** (4 * len(RESCALE_AFTER)))  # undo in final sqrt
OUT_SCALE = 2.0 ** -7         # fp16 output headroom; undone on host
SHRINK16 = 1.0
SHRINK32 = 1.0 - 2.0 ** -14

# f'-bit (layout2 free bit) paired by stage s
F_BIT = {}
for _s in range(13, 17):
    F_BIT[_s] = 3 + (_s - 13)      # global bits 12..15
for _s in range(17, 20):
    F_BIT[_s] = _s - 17            # global bits 16..18
for _s in range(20, 23):
    F_BIT[_s] = 9 + (_s - 20)      # global bits 19..21

TWL_OFF = {s: 2048 * (s - 13) for s in range(13, 17)}   # fp16 late table
TWL32_OFF = {s: 2048 * (s - 17) for s in range(17, 23)}  # fp32 late table

TWE_OFF = {}
_c = 0
for _s in range(2, 13):
    TWE_OFF[_s] = _c
    _c += 1 << (_s - 1)
TWE_COLS = _c


def _bitrev_perm():
    x = np.arange(N)
    r = np.zeros(N, dtype=np.int64)
    for b in range(NBITS):
        r = (r << 1) | ((x >> b) & 1)
    return r


def _fprime_to_n(fp, ww, d):
    """global index n from layout2 (core d, partition ww, free col f')."""
    return (ww
            + ((fp >> 0) & 7) * (1 << 16)
            + ((fp >> 3) & 15) * (1 << 12)
            + ((fp >> 7) & 3) * (1 << 7)
            + d * (1 << 9)
            + ((fp >> 9) & 7) * (1 << 19))


# --------------------------------------------------------------------------
# host-side tables / permutations (cached)
# --------------------------------------------------------------------------
_HOST_CACHE = {}


def _in_perm():
    """perm[d, w, f] -> index into x (full input)."""
    if "in" not in _HOST_CACHE:
        br = _bitrev_perm()
        w = np.arange(P)[:, None]
        f = np.arange(F)[None, :]
        l = f + ((w >> 3) & 15) * (1 << 12) + (w & 7) * (1 << 16)
        out = np.empty((M, P, F), dtype=np.int64)
        for d in range(M):
            out[d] = br[d * (1 << 19) + l]
        _HOST_CACHE["in"] = out
    return _HOST_CACHE["in"]


def _out_perm():
    """perm[d, ww, f'] -> global output index n."""
    if "out" not in _HOST_CACHE:
        ww = np.arange(P)[:, None]
        fp = np.arange(F)[None, :]
        out = np.empty((M, P, F), dtype=np.int64)
        for d in range(M):
            out[d] = _fprime_to_n(fp, ww, d)
        _HOST_CACHE["out"] = out
    return _HOST_CACHE["out"]


def host_tw_early():
    """[P, TWE_COLS] fp16: stages 2..12, cols 2^(s-1) each, identical rows."""
    if "twE" in _HOST_CACHE:
        return _HOST_CACHE["twE"]
    cols = []
    for s in range(2, 13):
        h = 1 << (s - 1)
        t = np.arange(h)
        cols.append(2.0 * np.cos(2 * np.pi * t / (1 << s)) * SHRINK16)
    row = np.concatenate(cols).astype(np.float16)
    _HOST_CACHE["twE"] = np.broadcast_to(row, (P, row.size)).copy()
    return _HOST_CACHE["twE"]


def _late_table(s, d, dtype, shrink):
    """[P, 2048] twiddle table for stage s, core d: col idx = f' with
    pairing bit F_BIT[s] removed; value 2*cos(2*pi*k/2^s)*shrink."""
    b = F_BIT[s]
    idx = np.arange(2048)
    fp = (idx & ((1 << b) - 1)) | ((idx >> b) << (b + 1))
    ww = np.arange(P)[:, None]
    n = _fprime_to_n(fp[None, :], ww, d)
    k = n % (1 << (s - 1))
    return (2.0 * np.cos(2 * np.pi * k / (1 << s)) * shrink).astype(dtype)


def host_tw_late16(d):
    key = ("twL16", d)
    if key not in _HOST_CACHE:
        _HOST_CACHE[key] = np.concatenate(
            [_late_table(s, d, np.float16, SHRINK16) for s in range(13, 17)],
            axis=1)
    return _HOST_CACHE[key]


def host_tw_late32(d):
    key = ("twL32", d)
    if key not in _HOST_CACHE:
        _HOST_CACHE[key] = np.concatenate(
            [_late_table(s, d, np.float32, SHRINK32) for s in range(17, 23)],
            axis=1)
    return _HOST_CACHE[key]


# --------------------------------------------------------------------------
# AP helpers
# --------------------------------------------------------------------------
def _ap(base, col_off, dims):
    """AP into a [P, C] tile at column offset with explicit free dims."""
    sl = base[:, col_off:] if col_off else base
    return AP(sl.tensor, sl.offset, [sl.ap[0]] + dims)


def _halve_strides(dims, b):
    """table AP dims from data AP dims when the table has pairing bit b
    removed: strides > 2^b halve; equal/lower stay."""
    thr = 1 << b
    return [[(st // 2 if st > thr else st), ct] for st, ct in dims]


def _compact(dims):
    """scratch-tile dims with the same counts, row-major compacted."""
    out = []
    acc = 1
    for st, ct in reversed(dims):
        out.insert(0, [acc, ct])
        acc *= ct
    return out


def _l2_dims(s):
    """(e-side free dims, e->o column delta) for a layout2 quarter: cols
    sblk*512 + u*128 + c relative to the u*128 base; stage s pairs f' bit
    b=F_BIT[s] (inside c for b<=6, inside sblk for b>=9)."""
    b = F_BIT[s]
    if b <= 6:
        lo = 1 << b
        hi = 64 // lo
        dims = [[512, 8]]
        if hi > 1:
            dims.append([2 * lo, hi])
        if lo > 1:
            dims.append([1, lo])
        return dims, lo
    j = b - 9
    lo = 1 << j
    hi = 4 // lo
    dims = []
    if hi > 1:
        dims.append([2 * lo * 512, hi])
    if lo > 1:
        dims.append([512, lo])
    dims.append([1, 128])
    return dims, lo * 512


def _cut(ap_, lo, hi, dim_idx):
    """restrict free dim dim_idx (0-based among free dims) to [lo, hi)."""
    dd = [list(x) for x in ap_.ap]
    st = dd[1 + dim_idx][0]
    dd[1 + dim_idx][1] = hi - lo
    return AP(ap_.tensor, ap_.offset + st * lo, dd)


# --------------------------------------------------------------------------
# device program
# --------------------------------------------------------------------------
def build_nc(stop_after=None, no_cc=False, NCH=4, O16=1.0, O32=0.27):
    nc = bacc.Bacc()

    x_in = nc.dram_tensor("x", [P, F], FP16, kind="ExternalInput")
    twe_in = nc.dram_tensor("twe", [P, TWE_COLS], FP16, kind="ExternalInput")
    twl16_in = nc.dram_tensor("twl16", [P, 8192], FP16, kind="ExternalInput")
    twl32_in = nc.dram_tensor("twl32", [P, 12288], FP32, kind="ExternalInput")
    ident_in = nc.dram_tensor("ident", [P, P], FP16, kind="ExternalInput")
    out = nc.dram_tensor("out", [P, F], FP16, kind="ExternalOutput")

    with tile.TileContext(nc) as tc:
        _handles = []
        sendh = []
        recvh = []
        for uh in range(4):
            s_t, _h1 = tc.tile([M, P, 128], FP16, space="DRAM",
                               name=f"a2a_send{uh}")
            r_t, _h2 = tc.tile([M, P, 128], FP16, space="DRAM",
                               addr_space="Shared", name=f"a2a_recv{uh}")
            _handles += [_h1, _h2]
            sendh.append(s_t)
            recvh.append(r_t)

        v16, _h3 = tc.tile([P, F], FP16, name="v16")     # layout1 state
        v2, _h4 = tc.tile([P, F], FP16, name="v2")       # layout2 fp16 state
        v32, _h5 = tc.tile([P, F], FP32, name="v32")     # layout2 fp32 state
        twe, _h6 = tc.tile([P, TWE_COLS], FP16, name="twe")
        twl16, _h7 = tc.tile([P, 8192], FP16, name="twl16")
        twl32, _h8 = tc.tile([P, 12288], FP32, name="twl32")
        ident_t, _h9 = tc.tile([P, P], FP16, name="ident")

        with (
            tc.tile_pool(name="io", bufs=1) as io_pool,
            tc.tile_pool(name="scratch", bufs=4) as sp,
            tc.tile_pool(name="psum", bufs=4, space="PSUM") as pp,
        ):
            xt = io_pool.tile([P, F], FP16, tag="io")
            x_engs = (nc.sync, nc.scalar, nc.gpsimd, nc.sync)
            for ci in range(4):
                x_engs[ci].dma_start(xt[:, ci * 1024:(ci + 1) * 1024],
                                     x_in[:, ci * 1024:(ci + 1) * 1024])
            # twe needed by stage 2 (~5us): cheap dispatch on the Pool queue.
            nc.gpsimd.dma_start(twe[:], twe_in[:])
            # The rest rides the otherwise-idle SP queue behind x.
            nc.sync.dma_start(ident_t[:], ident_in[:])
            for ci in range(2):
                nc.sync.dma_start(twl16[:, ci * 4096:(ci + 1) * 4096],
                                  twl16_in[:, ci * 4096:(ci + 1) * 4096])
            for ci in range(3):
                nc.sync.dma_start(twl32[:, ci * 4096:(ci + 1) * 4096],
                                  twl32_in[:, ci * 4096:(ci + 1) * 4096])

            def bail(label, ap, scale=1.0):
                """debug: dump fp16(scale*ap) and stop emitting stages."""
                if stop_after == label:
                    ot = io_pool.tile([P, F], FP16, tag="bailio")
                    nc.scalar.mul(ot[:], ap, scale)
                    nc.sync.dma_start(out[:], ot[:])
                    return True
                return False

            # ---------------- stage 1 (fp16 in -> fp16 squares) -----------
            xe = xt[:].rearrange("p (c pair) -> p c pair", pair=2)[:, :, 0]
            xo = xt[:].rearrange("p (c pair) -> p c pair", pair=2)[:, :, 1]
            ve = v16[:].rearrange("p (c pair) -> p c pair", pair=2)[:, :, 0]
            vo = v16[:].rearrange("p (c pair) -> p c pair", pair=2)[:, :, 1]
            s1 = []
            for c in range(4):
                sl = slice(c * 512, (c + 1) * 512)
                st = sp.tile([P, 512], FP16, tag="s1s")
                dt = sp.tile([P, 512], FP16, tag="s1d")
                nc.vector.tensor_tensor(st[:], xe[:, sl], xo[:, sl], OP.add)
                nc.gpsimd.tensor_tensor(dt[:], xe[:, sl], xo[:, sl],
                                        OP.subtract)
                s1.append((sl, st, dt))
            for (sl, st, dt) in s1:
                nc.vector.tensor_tensor(ve[:, sl], st[:], st[:], OP.mult)
                nc.scalar.activation(vo[:, sl], dt[:], AF.Square)
            stopped = bail('stage1', v16[:])

            # ---------------- full-width stages 2..9 ----------------------
            # chunk fractions: two small leading chunks so the first sqrt
            # returns before DVE drains its q-phase (no t2 stall)
            FRACS = (8, 1, 1, 2, 2, 2)   # denominator, then numerators
            def do_stage(s, rescale, nch):
                b = s - 1
                vv = v16[:].rearrange("p (o pair i) -> p o pair i",
                                      pair=2, i=1 << b)
                e_all, o_all = vv[:, :, 0, :], vv[:, :, 1, :]
                n_out = 1 << (11 - b)
                inner = 1 << b
                cd = []
                pos = 0
                for frac in FRACS[1:]:
                    nblk = n_out * frac // FRACS[0]
                    co = slice(pos, pos + nblk)
                    pos += nblk
                    ccols = nblk * inner
                    tw = _ap(twe[:], TWE_OFF[s], [[0, nblk], [1, inner]])
                    q = sp.tile([P, ccols], FP16, tag="q")
                    p32 = sp.tile([P, ccols], FP32, tag="p")
                    r2 = sp.tile([P, ccols], FP16, tag="r")
                    t2 = sp.tile([P, ccols], FP16, tag="t")
                    mk = lambda t_: t_.rearrange("p (o i) -> p o i", i=inner)
                    cd.append(dict(e=e_all[:, co, :], o=o_all[:, co, :],
                                   tw=tw, q=q, p32=p32, r2=r2, t2=t2,
                                   qv=mk(q[:]), pv=mk(p32[:]),
                                   rv=mk(r2[:]), tv=mk(t2[:])))
                rsc = RS if rescale else 1.0
                for c in cd:
                    nc.vector.tensor_tensor(c["qv"], c["e"], c["o"], OP.add)
                    nc.gpsimd.tensor_tensor(c["pv"], c["e"], c["o"], OP.mult)
                for c in cd:
                    nc.scalar.activation(c["r2"][:], c["p32"][:], AF.Sqrt,
                                         scale=rsc * rsc)
                    if rescale:
                        nc.vector.tensor_scalar(c["q"][:], c["q"][:], rsc,
                                                None, OP.mult)
                for c in cd:
                    nc.vector.tensor_tensor(c["tv"], c["tw"], c["rv"],
                                            OP.mult)
                    nc.vector.tensor_tensor(c["e"], c["qv"], c["tv"], OP.add)
                    if O16 >= 1.0 or inner < 16:
                        nc.vector.tensor_tensor(c["o"], c["qv"], c["tv"],
                                                OP.subtract)
                    else:
                        k = max(8, int(inner * O16)) & ~7
                        k = min(k, inner - 8)
                        nc.vector.tensor_tensor(
                            c["o"][:, :, 0:k], c["qv"][:, :, 0:k],
                            c["tv"][:, :, 0:k], OP.subtract)
                        nc.gpsimd.tensor_tensor(
                            c["o"][:, :, k:inner], c["qv"][:, :, k:inner],
                            c["tv"][:, :, k:inner], OP.subtract)

            def clamp_band(s):
                h = 1 << (s - 1)
                m = 2 * h
                g = max(1, h // 16)
                base = v16[:, h - g:]
                band = AP(base.tensor, base.offset,
                          [base.ap[0], [m, F // m], [1, 2 * g]])
                nc.vector.tensor_scalar(band, band, 0.0, None, OP.max)

            for s in range(2, 10):
                if stopped:
                    break
                do_stage(s, s in RESCALE_AFTER, NCH)
                clamp_band(s)
                stopped = stopped or bail(f'stage{s}', v16[:])

            # ------------- per-quarter stages 10..12 (layout1) ------------
            # phase-major across the 4 quarters (u = the chunk index)
            def stage_l1_allu(s, rescale, finish=None):
                j = s - 10                      # dp bit paired
                lo = 1 << j
                hi = 4 // lo
                dims = []
                tdims = []
                sdims = []
                if hi > 1:
                    dims.append([2 * lo * 512, hi])
                    tdims.append([0, hi])
                    sdims.append([lo * 128, hi])
                if lo > 1:
                    dims.append([512, lo])
                    tdims.append([512, lo])
                    sdims.append([128, lo])
                dims.append([1, 128])
                tdims.append([1, 128])
                sdims.append([1, 128])
                rsc = RS if rescale else 1.0
                cd = []
                for u in range(4):
                    q = sp.tile([P, 512], FP16, tag="q")
                    p32 = sp.tile([P, 512], FP32, tag="p")
                    r2 = sp.tile([P, 512], FP16, tag="r")
                    t2 = sp.tile([P, 512], FP16, tag="t")
                    cd.append(dict(
                        e=_ap(v16[:], u * 128, dims),
                        o=_ap(v16[:], u * 128 + lo * 512, dims),
                        tw=_ap(twe[:], TWE_OFF[s] + u * 128, tdims),
                        q=q, p32=p32, r2=r2, t2=t2,
                        qv=_ap(q[:], 0, sdims), pv=_ap(p32[:], 0, sdims),
                        rv=_ap(r2[:], 0, sdims), tv=_ap(t2[:], 0, sdims)))
                for c in cd:
                    nc.vector.tensor_tensor(c["qv"], c["e"], c["o"], OP.add)
                    nc.gpsimd.tensor_tensor(c["pv"], c["e"], c["o"], OP.mult)
                for c in cd:
                    nc.scalar.activation(c["r2"][:], c["p32"][:], AF.Sqrt,
                                         scale=rsc * rsc)
                    if rescale:
                        nc.vector.tensor_scalar(c["q"][:], c["q"][:], rsc,
                                                None, OP.mult)
                for u, c in enumerate(cd):
                    nc.vector.tensor_tensor(c["tv"], c["tw"], c["rv"],
                                            OP.mult)
                    nc.vector.tensor_tensor(c["e"], c["qv"], c["tv"], OP.add)
                    nc.vector.tensor_tensor(c["o"], c["qv"], c["tv"],
                                            OP.subtract)
                    clamp_band_l1u(s, u)
                    if finish is not None:
                        finish(u)

            def clamp_band_l1u(s, u):
                """layout1 clamp band (col mod 2h in [h-g, h+g)) restricted
                to quarter u, at 64-col granularity (over-clamp is safe)."""
                h = 1 << (s - 1)
                g = h // 16
                cols = []
                for dp in range(8):
                    base = dp * 512 + u * 128
                    for cb in (base, base + 64):
                        kk = cb % (2 * h)
                        if kk + 64 > h - g and kk < h + g:
                            cols.append(cb)
                i = 0
                while i < len(cols):
                    jf = i
                    while jf + 1 < len(cols) and cols[jf + 1] == cols[jf] + 64:
                        jf += 1
                    band = v16[:, cols[i]:cols[jf] + 64]
                    nc.vector.tensor_scalar(band, band, 0.0, None, OP.max)
                    i = jf + 1

            # ------------- layout2 stage helper (phase-major over u) ------
            def stage_l2_allu(s, fp16, o_dve_frac, in16=None, finish=None):
                b = F_BIT[s]
                dims, d_pair = _l2_dims(s)
                dt_state = FP16 if fp16 else FP32
                vt = v2 if fp16 else v32
                vin = v2 if (fp16 or in16) else v32
                tw_t = twl16 if s <= 16 else twl32
                t0 = (TWL_OFF if s <= 16 else TWL32_OFF)[s]
                sdims = _compact(dims)
                cd = []
                for u in range(4):
                    q = sp.tile([P, 512], dt_state, tag="q")
                    p32 = sp.tile([P, 512], FP32, tag="p")
                    r2 = sp.tile([P, 512], dt_state, tag="r")
                    t2 = sp.tile([P, 512], dt_state, tag="t")
                    toff = u * (64 if b < 7 else 128)
                    cd.append(dict(
                        ein=_ap(vin[:], u * 128, dims),
                        oin=_ap(vin[:], u * 128 + d_pair, dims),
                        e=_ap(vt[:], u * 128, dims),
                        o=_ap(vt[:], u * 128 + d_pair, dims),
                        tw=_ap(tw_t[:], t0 + toff, _halve_strides(dims, b)),
                        q=q, p32=p32, r2=r2, t2=t2,
                        qv=_ap(q[:], 0, sdims), pv=_ap(p32[:], 0, sdims),
                        rv=_ap(r2[:], 0, sdims), tv=_ap(t2[:], 0, sdims)))
                for c in cd:
                    nc.vector.tensor_tensor(c["qv"], c["ein"], c["oin"],
                                            OP.add)
                    nc.gpsimd.tensor_tensor(c["pv"], c["ein"], c["oin"],
                                            OP.mult)
                for c in cd:
                    nc.scalar.activation(c["r2"][:], c["p32"][:], AF.Sqrt)
                for u, c in enumerate(cd):
                    o, qv, tv = c["o"], c["qv"], c["tv"]
                    nc.vector.tensor_tensor(tv, c["tw"], c["rv"], OP.mult)
                    nc.vector.tensor_tensor(c["e"], qv, tv, OP.add)
                    if o_dve_frac >= 1.0:
                        nc.vector.tensor_tensor(o, qv, tv, OP.subtract)
                    else:
                        di = max(range(len(dims)),
                                 key=lambda i: dims[i][1])
                        ct = dims[di][1]
                        k = max(1, min(ct - 1, round(ct * o_dve_frac)))
                        nc.vector.tensor_tensor(
                            _cut(o, 0, k, di), _cut(qv, 0, k, di),
                            _cut(tv, 0, k, di), OP.subtract)
                        nc.gpsimd.tensor_tensor(
                            _cut(o, k, ct, di), _cut(qv, k, ct, di),
                            _cut(tv, k, ct, di), OP.subtract)
                    if fp16 and s <= 16:
                        clamp_band2_u(s, u)
                        if s in RESCALE_AFTER:
                            for halfc in range(2):
                                slv = _ap(v2[:], u * 128 + halfc * 2048,
                                          [[512, 4], [1, 128]])
                                nc.vector.tensor_scalar(slv, slv, RS, None,
                                                        OP.mult)
                    if finish is not None:
                        finish(u)

            # layout2 fp16 cancellation clamps, restricted to quarter u.
            CLAMP2 = {
                13: [(8, [[512, 8], [16, 8], [1, 8]]),
                     (384, [[512, 8], [16, 8], [1, 8]])],
                14: [(16, [[512, 8], [32, 4], [1, 8]]),
                     (392, [[512, 8], [32, 4], [1, 8]])],
                15: [(32, [[512, 8], [64, 4], [1, 8]]),
                     (280, [[512, 8], [64, 4], [1, 8]])],
                16: [(64, [[128, 32], [1, 8]]),
                     (56, [[128, 32], [1, 8]])],
            }

            def clamp_band2_u(s, u):
                for off, dims in CLAMP2[s]:
                    # dims whose span stays inside one 128-col block keep
                    # their AP form; the rest are enumerated so cols can be
                    # quarter-filtered (u = col bits 7..8).
                    hi = [dx for dx in dims
                          if dx[0] >= 128 or dx[0] * dx[1] > 128]
                    lo = [dx for dx in dims if dx not in hi]
                    combos = [0]
                    for stx, ctx in hi:
                        combos = [c0 + stx * i for c0 in combos
                                  for i in range(ctx)]
                    cols = sorted(c0 + off for c0 in combos
                                  if ((c0 + off) >> 7) & 3 == u)
                    groups = {}
                    for c0 in cols:
                        groups.setdefault(c0 % 512, []).append(c0)
                    for gcols in groups.values():
                        if len(gcols) > 1:
                            stq = gcols[1] - gcols[0]
                            assert all(gcols[i + 1] - gcols[i] == stq
                                       for i in range(len(gcols) - 1)), \
                                (s, gcols)
                            bdims = [[stq, len(gcols)]] + lo
                        else:
                            bdims = lo if lo else [[1, 1]]
                        band = _ap(v2[:], gcols[0], bdims)
                        nc.vector.tensor_scalar(band, band, 0.0, None,
                                                OP.max)

            # ------------- per-quarter pipeline ---------------------------
            # transpose + send + a2a + recv for one quarter (chained onto
            # the stage-12 epilogue so quarter u ships while u+1 computes)
            def ship(u):
                stg = sp.tile([P, 1024], FP16, tag="stg")
                for bi in range(0, M, 4):
                    pt = pp.tile([P, 512], FP16, tag="pt")
                    for j, dp in enumerate(range(bi, bi + 4)):
                        c0 = dp * 512 + u * 128
                        nc.tensor.transpose(pt[:, j * 128:(j + 1) * 128],
                                            v16[:, c0:c0 + 128], ident_t[:])
                    dst = stg[:, bi * 128:(bi + 4) * 128]
                    if bi == 0:
                        nc.vector.tensor_scalar(dst, pt[:], 1.0, None,
                                                OP.mult)
                    else:
                        nc.scalar.copy(dst, pt[:])
                nc.sync.dma_start(
                    sendh[u][:].rearrange("d w c -> w d c"),
                    stg[:].rearrange("w (d c) -> w d c", d=M))
                if no_cc:
                    # transport stand-in; ACT's next compute (stage-13
                    # sqrts) waits on the recvs regardless
                    nc.scalar.dma_start(recvh[u][:], sendh[u][:])
                else:
                    nc.gpsimd.collective_compute(
                        "AllToAll", OP.bypass,
                        replica_groups=[list(range(M))],
                        ins=[sendh[u][:].opt()],
                        outs=[recvh[u][:].opt()])
                if u == 3:
                    # recvs go last on SP so they never head-block sends
                    # (a recv waiting on its collective would stall any
                    # queue it shares with later compute or sends).
                    for ur in range(4):
                        nc.sync.dma_start(
                            v2[:].rearrange("w (s u c) -> w s u c",
                                            s=M, u=4)[:, :, ur, :],
                            recvh[ur][:].rearrange("s w c -> w s c"))

            # final magnitudes + output DMA for one quarter
            def final(u):
                ot = sp.tile([P, 1024], FP16, tag="ot")
                nc.scalar.activation(
                    _ap(ot[:], 0, [[128, 8], [1, 128]]),
                    _ap(v32[:], u * 128, [[512, 8], [1, 128]]),
                    AF.Sqrt, scale=FINAL_SCALE * OUT_SCALE * OUT_SCALE)
                eng = nc.sync if u % 2 == 0 else nc.scalar
                eng.dma_start(
                    out[:].rearrange("w (s u c) -> w s u c",
                                     s=M, u=4)[:, :, u, :],
                    ot[:].rearrange("w (s c) -> w s c", s=M))

            if not stopped:
                stage_l1_allu(10, 10 in RESCALE_AFTER)
                stage_l1_allu(11, False)
                stage_l1_allu(12, 12 in RESCALE_AFTER, finish=ship)
                stopped = bail('a2a', v2[:])
            for s in range(13, 17):
                if stopped:
                    break
                stage_l2_allu(s, True, 1.0)
                stopped = stopped or bail(f'stage{s}', v2[:])
            if not stopped:
                stage_l2_allu(17, False, 0.5, in16=True)
                for s in range(18, 22):
                    stage_l2_allu(s, False, O32)
                stage_l2_allu(22, False, O32, finish=final)

    nc.finalize()
    return nc


_NC_CACHE = None


def _get_nc():
    global _NC_CACHE
    if _NC_CACHE is None:
        _NC_CACHE = build_nc()
    return _NC_CACHE


def host_inputs(x):
    perm = _in_perm()
    xv = x[perm].astype(np.float16)          # [M, P, F] fp16
    ident = np.eye(P, dtype=np.float16)
    twe = host_tw_early()
    return [dict(x=xv[d], twe=twe, twl16=host_tw_late16(d),
                 twl32=host_tw_late32(d), ident=ident) for d in range(M)]


def assemble(outs):
    operm = _out_perm()
    full = np.empty(N, dtype=np.float32)
    inv = np.float32(1.0 / OUT_SCALE)
    for d in range(M):
        full[operm[d].reshape(-1)] = (
            np.asarray(outs[d]).astype(np.float32).reshape(-1) * inv)
    return full


def kernel(x: np.ndarray) -> np.ndarray:
    x = np.asarray(x)
    assert x.shape == (N,) and x.dtype == np.float32, (x.shape, x.dtype)
    in_maps = host_inputs(x)
    nc = _get_nc()
    res = run_bass_kernel_spmd(nc, in_maps, core_ids=list(range(M)))
    return assemble([res.results[d]["out"] for d in range(M)])


if __name__ == "__main__":
    rng = np.random.default_rng(0)
    x = rng.standard_normal(N).astype(np.float32)
    r = kernel(x)
    print("kernel ran, out[:4] =", r[:4])


# revision 34
# speedup vs baseline: 1.1071x; 1.0004x over previous
"""Distributed 2^22-point radix-2 FFT-with-abs-at-every-stage on 8 NeuronCores.

Math: the reference applies abs() after every butterfly stage, so all state is
real and non-negative.  We propagate SQUARED magnitudes v = |y|^2:
    stage s:  q = v_e + v_o ;  r = sqrt(v_e * v_o)
              v_e' = q + 2*cos(2*pi*k/2^s)*r ;  v_o' = q - 2*c*r
(no sines needed: |e + (c - i s) o|^2 = e^2 + o^2 + 2 c e o for e,o >= 0).

Precision: stages 1..16 fp16 (DVE 2x_1p packing), 17..22 fp32.  Flat x2^-4
rescales after stages 6,8,10,12,14 keep fp16 in range; the product (2^-20)
is undone in the final sqrt's free ACT scale.  Rescales fold into the stage
(q scaled by one TS op, r scaled inside the ACT sqrt) - never a separate
pass.  Input is cast fp16 on host; output is fp16 scaled by 2^-7 and upcast
on host (power-of-2 scales are error-free).

Distribution (one all-to-all; butterflies pair along the free dimension):
  - host bit-reverses x and shards contiguously: core d owns bits 21..19 = d
  - layout1 [128, 4096]: partition w = permuted bits 18..12
    (w = [b16 b17 b18 | b12 b13 b14 b15], LSB first), free f = bits 11..0
    -> stages 1..12 pair free bits 0..11
  - PE-transpose + AllToAll resharding -> layout2 [128, 4096]:
    partition ww = bits 6..0, f' = [b16 b17 b18 | b12..b15 | b7 b8 | b19..b21]
    -> stages 13..16 pair f' bits 3..6 (fp16), 17..19 pair f' 0..2 (fp32),
       20..22 pair f' 9..11 (fp32).
Pipelining: f bits 7..8 (= f' bits 7..8 = the a2a quarter index u) are last
paired at stage 9, so stages 10..22 + transpose + a2a + final output all run
per-quarter: a 4-deep pipeline that hides the collective and output DMA.
Engines: DVE: q, t2, e', most of o'; Pool: p = v_e*v_o (fp32 out), rest of
o'; ACT: r = sqrt(p) (+ folded rescale), PSUM drains, final sqrt.
"""

import numpy as np

import concourse.bacc as bacc
import concourse.mybir as mybir
import concourse.tile as tile
from concourse.bass_types import AP
from concourse.bass_utils import run_bass_kernel_spmd

FP32 = mybir.dt.float32
FP16 = mybir.dt.float16
AF = mybir.ActivationFunctionType
OP = mybir.AluOpType

NBITS = 22
N = 1 << NBITS
M = 8          # cores
P = 128
F = 4096

RESCALE_AFTER = (6, 8, 10, 12, 14)
RS = 2.0 ** -4
FINAL_SCALE = float(2.0 ** (4 * len(RESCALE_AFTER)))  # undo in final sqrt
OUT_SCALE = 2.0 ** -7         # fp16 output headroom; undone on host
SHRINK16 = 1.0
SHRINK32 = 1.0 - 2.0 ** -14

# f'-bit (layout2 free bit) paired by stage s
F_BIT = {}
for _s in range(13, 17):
    F_BIT[_s] = 3 + (_s - 13)      # global bits 12..15
for _s in range(17, 20):
    F_BIT[_s] = _s - 17            # global bits 16..18
for _s in range(20, 23):
    F_BIT[_s] = 9 + (_s - 20)      # global bits 19..21

TWL_OFF = {s: 2048 * (s - 13) for s in range(13, 17)}   # fp16 late table
TWL32_OFF = {s: 2048 * (s - 17) for s in range(17, 23)}  # fp32 late table

TWE_OFF = {}
_c = 0
for _s in range(2, 13):
    TWE_OFF[_s] = _c
    _c += 1 << (_s - 1)
TWE_COLS = _c


def _bitrev_perm():
    x = np.arange(N)
    r = np.zeros(N, dtype=np.int64)
    for b in range(NBITS):
        r = (r << 1) | ((x >> b) & 1)
    return r


def _fprime_to_n(fp, ww, d):
    """global index n from layout2 (core d, partition ww, free col f')."""
    return (ww
            + ((fp >> 0) & 7) * (1 << 16)
            + ((fp >> 3) & 15) * (1 << 12)
            + ((fp >> 7) & 3) * (1 << 7)
            + d * (1 << 9)
            + ((fp >> 9) & 7) * (1 << 19))


# --------------------------------------------------------------------------
# host-side tables / permutations (cached)
# --------------------------------------------------------------------------
_HOST_CACHE = {}


def _in_perm():
    """perm[d, w, f] -> index into x (full input)."""
    if "in" not in _HOST_CACHE:
        br = _bitrev_perm()
        w = np.arange(P)[:, None]
        f = np.arange(F)[None, :]
        l = f + ((w >> 3) & 15) * (1 << 12) + (w & 7) * (1 << 16)
        out = np.empty((M, P, F), dtype=np.int64)
        for d in range(M):
            out[d] = br[d * (1 << 19) + l]
        _HOST_CACHE["in"] = out
    return _HOST_CACHE["in"]


def _out_perm():
    """perm[d, ww, f'] -> global output index n."""
    if "out" not in _HOST_CACHE:
        ww = np.arange(P)[:, None]
        fp = np.arange(F)[None, :]
        out = np.empty((M, P, F), dtype=np.int64)
        for d in range(M):
            out[d] = _fprime_to_n(fp, ww, d)
        _HOST_CACHE["out"] = out
    return _HOST_CACHE["out"]


def host_tw_early():
    """[P, TWE_COLS] fp16: stages 2..12, cols 2^(s-1) each, identical rows.
    Rescale-stage slices are pre-scaled by RS (the q path applies RS with
    one TS op; t2 = tw*r picks it up from the table)."""
    if "twE" in _HOST_CACHE:
        return _HOST_CACHE["twE"]
    cols = []
    for s in range(2, 13):
        h = 1 << (s - 1)
        t = np.arange(h)
        rsc = RS if s in RESCALE_AFTER else 1.0
        cols.append(2.0 * np.cos(2 * np.pi * t / (1 << s)) * SHRINK16 * rsc)
    row = np.concatenate(cols).astype(np.float16)
    _HOST_CACHE["twE"] = np.broadcast_to(row, (P, row.size)).copy()
    return _HOST_CACHE["twE"]


def _late_table(s, d, dtype, shrink):
    """[P, 2048] twiddle table for stage s, core d: col idx = f' with
    pairing bit F_BIT[s] removed; value 2*cos(2*pi*k/2^s)*shrink."""
    b = F_BIT[s]
    idx = np.arange(2048)
    fp = (idx & ((1 << b) - 1)) | ((idx >> b) << (b + 1))
    ww = np.arange(P)[:, None]
    n = _fprime_to_n(fp[None, :], ww, d)
    k = n % (1 << (s - 1))
    return (2.0 * np.cos(2 * np.pi * k / (1 << s)) * shrink).astype(dtype)


def host_tw_late16(d):
    key = ("twL16", d)
    if key not in _HOST_CACHE:
        _HOST_CACHE[key] = np.concatenate(
            [_late_table(s, d, np.float16, SHRINK16) *
             np.float16(RS if s in RESCALE_AFTER else 1.0)
             for s in range(13, 17)], axis=1)
    return _HOST_CACHE[key]


def host_tw_late32(d):
    key = ("twL32", d)
    if key not in _HOST_CACHE:
        _HOST_CACHE[key] = np.concatenate(
            [_late_table(s, d, np.float32, SHRINK32) for s in range(17, 23)],
            axis=1)
    return _HOST_CACHE[key]


# --------------------------------------------------------------------------
# AP helpers
# --------------------------------------------------------------------------
def _ap(base, col_off, dims):
    """AP into a [P, C] tile at column offset with explicit free dims."""
    sl = base[:, col_off:] if col_off else base
    return AP(sl.tensor, sl.offset, [sl.ap[0]] + dims)


def _halve_strides(dims, b):
    """table AP dims from data AP dims when the table has pairing bit b
    removed: strides > 2^b halve; equal/lower stay."""
    thr = 1 << b
    return [[(st // 2 if st > thr else st), ct] for st, ct in dims]


def _compact(dims):
    """scratch-tile dims with the same counts, row-major compacted."""
    out = []
    acc = 1
    for st, ct in reversed(dims):
        out.insert(0, [acc, ct])
        acc *= ct
    return out


def _l2_dims(s):
    """(e-side free dims, e->o column delta) for a layout2 quarter: cols
    sblk*512 + u*128 + c relative to the u*128 base; stage s pairs f' bit
    b=F_BIT[s] (inside c for b<=6, inside sblk for b>=9)."""
    b = F_BIT[s]
    if b <= 6:
        lo = 1 << b
        hi = 64 // lo
        dims = [[512, 8]]
        if hi > 1:
            dims.append([2 * lo, hi])
        if lo > 1:
            dims.append([1, lo])
        return dims, lo
    j = b - 9
    lo = 1 << j
    hi = 4 // lo
    dims = []
    if hi > 1:
        dims.append([2 * lo * 512, hi])
    if lo > 1:
        dims.append([512, lo])
    dims.append([1, 128])
    return dims, lo * 512


def _cut(ap_, lo, hi, dim_idx):
    """restrict free dim dim_idx (0-based among free dims) to [lo, hi)."""
    dd = [list(x) for x in ap_.ap]
    st = dd[1 + dim_idx][0]
    dd[1 + dim_idx][1] = hi - lo
    return AP(ap_.tensor, ap_.offset + st * lo, dd)


# --------------------------------------------------------------------------
# device program
# --------------------------------------------------------------------------
def build_nc(stop_after=None, no_cc=False, NCH=4, O16=1.0, O32=0.27):
    nc = bacc.Bacc()

    x_in = nc.dram_tensor("x", [P, F], FP16, kind="ExternalInput")
    twe_in = nc.dram_tensor("twe", [P, TWE_COLS], FP16, kind="ExternalInput")
    twl16_in = nc.dram_tensor("twl16", [P, 8192], FP16, kind="ExternalInput")
    twl32_in = nc.dram_tensor("twl32", [P, 12288], FP32, kind="ExternalInput")
    ident_in = nc.dram_tensor("ident", [P, P], FP16, kind="ExternalInput")
    out = nc.dram_tensor("out", [P, F], FP16, kind="ExternalOutput")

    with tile.TileContext(nc) as tc:
        _handles = []
        sendh = []
        recvh = []
        for uh in range(4):
            s_t, _h1 = tc.tile([M, P, 128], FP16, space="DRAM",
                               name=f"a2a_send{uh}")
            r_t, _h2 = tc.tile([M, P, 128], FP16, space="DRAM",
                               addr_space="Shared", name=f"a2a_recv{uh}")
            _handles += [_h1, _h2]
            sendh.append(s_t)
            recvh.append(r_t)

        v16, _h3 = tc.tile([P, F], FP16, name="v16")     # layout1 state
        v2, _h4 = tc.tile([P, F], FP16, name="v2")       # layout2 fp16 state
        v32, _h5 = tc.tile([P, F], FP32, name="v32")     # layout2 fp32 state
        u16, _hu1 = tc.tile([P, F], FP16, name="u16")    # sqrt(v16) state
        u2, _hu2 = tc.tile([P, F], FP16, name="u2")      # sqrt(v2) state
        twe, _h6 = tc.tile([P, TWE_COLS], FP16, name="twe")
        twl16, _h7 = tc.tile([P, 8192], FP16, name="twl16")
        twl32, _h8 = tc.tile([P, 12288], FP32, name="twl32")
        ident_t, _h9 = tc.tile([P, P], FP16, name="ident")

        with (
            tc.tile_pool(name="io", bufs=1) as io_pool,
            tc.tile_pool(name="scratch", bufs=4) as sp,
            tc.tile_pool(name="psum", bufs=4, space="PSUM") as pp,
        ):
            xt = io_pool.tile([P, F], FP16, tag="io")
            x_engs = (nc.sync, nc.scalar, nc.gpsimd, nc.sync)
            for ci in range(4):
                x_engs[ci].dma_start(xt[:, ci * 1024:(ci + 1) * 1024],
                                     x_in[:, ci * 1024:(ci + 1) * 1024])
            # twe needed by stage 2 (~5us): cheap dispatch on the Pool queue.
            nc.gpsimd.dma_start(twe[:], twe_in[:])
            # The rest rides the otherwise-idle SP queue behind x.
            nc.sync.dma_start(ident_t[:], ident_in[:])
            for ci in range(2):
                nc.sync.dma_start(twl16[:, ci * 4096:(ci + 1) * 4096],
                                  twl16_in[:, ci * 4096:(ci + 1) * 4096])
            for ci in range(3):
                nc.sync.dma_start(twl32[:, ci * 4096:(ci + 1) * 4096],
                                  twl32_in[:, ci * 4096:(ci + 1) * 4096])

            def bail(label, ap, scale=1.0):
                """debug: dump fp16(scale*ap) and stop emitting stages."""
                if stop_after == label:
                    ot = io_pool.tile([P, F], FP16, tag="bailio")
                    nc.scalar.mul(ot[:], ap, scale)
                    nc.sync.dma_start(out[:], ot[:])
                    return True
                return False

            # ---------------- stage 1 (fp16 in -> fp16 squares) -----------
            xe = xt[:].rearrange("p (c pair) -> p c pair", pair=2)[:, :, 0]
            xo = xt[:].rearrange("p (c pair) -> p c pair", pair=2)[:, :, 1]
            ve = v16[:].rearrange("p (c pair) -> p c pair", pair=2)[:, :, 0]
            vo = v16[:].rearrange("p (c pair) -> p c pair", pair=2)[:, :, 1]
            ue = u16[:].rearrange("p (c pair) -> p c pair", pair=2)[:, :, 0]
            uo = u16[:].rearrange("p (c pair) -> p c pair", pair=2)[:, :, 1]
            s1 = []
            for c in range(4):
                sl = slice(c * 512, (c + 1) * 512)
                st = sp.tile([P, 512], FP16, tag="s1s")
                dt = sp.tile([P, 512], FP16, tag="s1d")
                nc.vector.tensor_tensor(st[:], xe[:, sl], xo[:, sl], OP.add)
                nc.gpsimd.tensor_tensor(dt[:], xe[:, sl], xo[:, sl],
                                        OP.subtract)
                s1.append((sl, st, dt))
            for (sl, st, dt) in s1:
                nc.vector.tensor_tensor(ve[:, sl], st[:], st[:], OP.mult)
                nc.vector.tensor_tensor(vo[:, sl], dt[:], dt[:], OP.mult)
                nc.scalar.activation(ue[:, sl], ve[:, sl], AF.Sqrt)
                nc.scalar.activation(uo[:, sl], vo[:, sl], AF.Sqrt)
            stopped = bail('stage1', v16[:])

            # ---------------- full-width stages 2..9 ----------------------
            # u-state form: r = u_e*u_o on DVE (no Pool product / ACT sqrt
            # on the critical path); ACT computes the two OUTPUT sqrts that
            # feed the next stage.  Chunk fractions: small lead chunk so
            # the pipeline primes, small tail chunk so the next stage's
            # first sqrt lands early.
            def do_stage(s, rescale, nch):
                b = s - 1
                vv = v16[:].rearrange("p (o pair i) -> p o pair i",
                                      pair=2, i=1 << b)
                uu = u16[:].rearrange("p (o pair i) -> p o pair i",
                                      pair=2, i=1 << b)
                e_all, o_all = vv[:, :, 0, :], vv[:, :, 1, :]
                ue_all, uo_all = uu[:, :, 0, :], uu[:, :, 1, :]
                n_out = 1 << (11 - b)
                inner = 1 << b
                nblk = n_out // 4
                cd = []
                for ci in range(4):
                    co = slice(ci * nblk, (ci + 1) * nblk)
                    ccols = nblk * inner
                    tw = _ap(twe[:], TWE_OFF[s], [[0, nblk], [1, inner]])
                    q = sp.tile([P, ccols], FP16, tag="q")
                    r2 = sp.tile([P, ccols], FP16, tag="r")
                    t2 = sp.tile([P, ccols], FP16, tag="t")
                    mk = lambda t_: t_.rearrange("p (o i) -> p o i", i=inner)
                    cd.append(dict(e=e_all[:, co, :], o=o_all[:, co, :],
                                   uei=ue_all[:, co, :],
                                   uoi=uo_all[:, co, :],
                                   tw=tw, q=q, r2=r2, t2=t2,
                                   qv=mk(q[:]), rv=mk(r2[:]), tv=mk(t2[:])))
                rsc = RS if rescale else 1.0
                # q on Pool (input-only dep: starts at stage begin);
                # r/t2/e'/o' on DVE; clamps per chunk; sqrts per half.
                for c in cd:
                    nc.gpsimd.tensor_tensor(c["qv"], c["e"], c["o"], OP.add)
                    nc.vector.tensor_tensor(c["rv"], c["uei"], c["uoi"],
                                            OP.mult)
                for ci, c in enumerate(cd):
                    if rescale:
                        nc.vector.tensor_scalar(c["q"][:], c["q"][:], rsc,
                                                None, OP.mult)
                    nc.vector.tensor_tensor(c["tv"], c["tw"], c["rv"],
                                            OP.mult)
                    nc.vector.tensor_tensor(c["e"], c["qv"], c["tv"], OP.add)
                    nc.vector.tensor_tensor(c["o"], c["qv"], c["tv"],
                                            OP.subtract)
                    clamp_band(s, ci * nblk, nblk)
                    if ci % 2 == 1:
                        a, bnd = (ci - 1) * nblk, (ci + 1) * nblk
                        nc.scalar.activation(ue_all[:, a:bnd, :],
                                             e_all[:, a:bnd, :], AF.Sqrt)
                        nc.scalar.activation(uo_all[:, a:bnd, :],
                                             o_all[:, a:bnd, :], AF.Sqrt)

            def clamp_band(s, blk0, nblk):
                """max(v,0) on the cancellation band, outer blocks
                [blk0, blk0+nblk) only."""
                h = 1 << (s - 1)
                m = 2 * h
                g = max(1, h // 16)
                base = v16[:, blk0 * m + h - g:]
                band = AP(base.tensor, base.offset,
                          [base.ap[0], [m, nblk], [1, 2 * g]])
                nc.vector.tensor_scalar(band, band, 0.0, None, OP.max)

            for s in range(2, 10):
                if stopped:
                    break
                do_stage(s, s in RESCALE_AFTER, NCH)
                stopped = stopped or bail(f'stage{s}', v16[:])

            # ------------- per-quarter stages 10..12 (layout1) ------------
            # phase-major across the 4 quarters (u = the chunk index)
            def stage_l1_allu(s, rescale, finish=None):
                j = s - 10                      # dp bit paired
                lo = 1 << j
                hi = 4 // lo
                dims = []
                tdims = []
                sdims = []
                if hi > 1:
                    dims.append([2 * lo * 512, hi])
                    tdims.append([0, hi])
                    sdims.append([lo * 128, hi])
                if lo > 1:
                    dims.append([512, lo])
                    tdims.append([512, lo])
                    sdims.append([128, lo])
                dims.append([1, 128])
                tdims.append([1, 128])
                sdims.append([1, 128])
                rsc = RS if rescale else 1.0
                last = s == 12
                cd = []
                for u in range(4):
                    q = sp.tile([P, 512], FP16, tag="q")
                    r2 = sp.tile([P, 512], FP16, tag="r")
                    t2 = sp.tile([P, 512], FP16, tag="t")
                    cd.append(dict(
                        u=u,
                        e=_ap(v16[:], u * 128, dims),
                        o=_ap(v16[:], u * 128 + lo * 512, dims),
                        ue=_ap(u16[:], u * 128, dims),
                        uo=_ap(u16[:], u * 128 + lo * 512, dims),
                        tw=_ap(twe[:], TWE_OFF[s] + u * 128, tdims),
                        qv=_ap(q[:], 0, sdims), q=q,
                        rv=_ap(r2[:], 0, sdims), tv=_ap(t2[:], 0, sdims)))
                for c in cd:
                    nc.gpsimd.tensor_tensor(c["qv"], c["e"], c["o"], OP.add)
                    nc.vector.tensor_tensor(c["rv"], c["ue"], c["uo"],
                                            OP.mult)
                for c in cd:
                    if rescale:
                        nc.vector.tensor_scalar(c["q"][:], c["q"][:], rsc,
                                                None, OP.mult)
                    nc.vector.tensor_tensor(c["tv"], c["tw"], c["rv"],
                                            OP.mult)
                    nc.vector.tensor_tensor(c["e"], c["qv"], c["tv"], OP.add)
                    nc.vector.tensor_tensor(c["o"], c["qv"], c["tv"],
                                            OP.subtract)
                    clamp_band_l1u(s, c["u"])
                    if not last:
                        nc.scalar.activation(c["ue"], c["e"], AF.Sqrt)
                        nc.scalar.activation(c["uo"], c["o"], AF.Sqrt)
                    if finish is not None:
                        finish(c["u"])

            def clamp_band_l1u(s, u):
                """layout1 clamp band (col mod 2h in [h-g, h+g)) restricted
                to quarter u, at 64-col granularity (over-clamp is safe)."""
                h = 1 << (s - 1)
                g = h // 16
                cols = []
                for dp in range(8):
                    base = dp * 512 + u * 128
                    for cb in (base, base + 64):
                        kk = cb % (2 * h)
                        if kk + 64 > h - g and kk < h + g:
                            cols.append(cb)
                i = 0
                while i < len(cols):
                    jf = i
                    while jf + 1 < len(cols) and cols[jf + 1] == cols[jf] + 64:
                        jf += 1
                    band = v16[:, cols[i]:cols[jf] + 64]
                    nc.vector.tensor_scalar(band, band, 0.0, None, OP.max)
                    i = jf + 1

            # ------------- layout2 stage helper (phase-major over u) ------
            def stage_l2_allu(s, fp16, o_dve_frac, in16=None, finish=None):
                b = F_BIT[s]
                dims, d_pair = _l2_dims(s)
                dt_state = FP16 if fp16 else FP32
                vt = v2 if fp16 else v32
                vin = v2 if (fp16 or in16) else v32
                tw_t = twl16 if s <= 16 else twl32
                t0 = (TWL_OFF if s <= 16 else TWL32_OFF)[s]
                sdims = _compact(dims)
                di = max(range(len(dims)), key=lambda i: dims[i][1])
                dct = dims[di][1]
                cd = []
                for u in range(4):
                    q = sp.tile([P, 512], dt_state, tag="q")
                    p32 = sp.tile([P, 512], FP32, tag="p")
                    r2 = sp.tile([P, 512], dt_state, tag="r")
                    t2 = sp.tile([P, 512], dt_state, tag="t")
                    toff = u * (64 if b < 7 else 128)
                    full = dict(
                        ein=_ap(vin[:], u * 128, dims),
                        oin=_ap(vin[:], u * 128 + d_pair, dims),
                        e=_ap(vt[:], u * 128, dims),
                        o=_ap(vt[:], u * 128 + d_pair, dims),
                        ue=_ap(u2[:], u * 128, dims),
                        uo=_ap(u2[:], u * 128 + d_pair, dims),
                        tw=_ap(tw_t[:], t0 + toff, _halve_strides(dims, b)),
                        qv=_ap(q[:], 0, sdims), pv=_ap(p32[:], 0, sdims),
                        rv=_ap(r2[:], 0, sdims), tv=_ap(t2[:], 0, sdims))
                    if u == 0 and not fp16:
                        h = dct // 2
                        for (a, bnd) in ((0, h), (h, dct)):
                            cc = {k: _cut(v, a, bnd, di)
                                  for k, v in full.items()}
                            cc["u"] = u if bnd == dct else None
                            cd.append(cc)
                    else:
                        full["u"] = u
                        cd.append(full)
                for c in cd:
                    if fp16:
                        nc.gpsimd.tensor_tensor(c["qv"], c["ein"],
                                                c["oin"], OP.add)
                        nc.vector.tensor_tensor(c["rv"], c["ue"], c["uo"],
                                                OP.mult)
                    else:
                        nc.vector.tensor_tensor(c["qv"], c["ein"],
                                                c["oin"], OP.add)
                        nc.gpsimd.tensor_tensor(c["pv"], c["ein"],
                                                c["oin"], OP.mult)
                if not fp16:
                    for c in cd:
                        nc.scalar.activation(c["rv"], c["pv"], AF.Sqrt)
                for c in cd:
                    o, qv, tv = c["o"], c["qv"], c["tv"]
                    if fp16 and s in RESCALE_AFTER:
                        nc.vector.tensor_scalar(qv, qv, RS, None, OP.mult)
                    nc.vector.tensor_tensor(tv, c["tw"], c["rv"], OP.mult)
                    nc.vector.tensor_tensor(c["e"], qv, tv, OP.add)
                    cdims = [list(x) for x in o.ap[1:]]
                    if fp16 or o_dve_frac >= 1.0:
                        nc.vector.tensor_tensor(o, qv, tv, OP.subtract)
                    else:
                        dj = max(range(len(cdims)),
                                 key=lambda i: cdims[i][1])
                        ct = cdims[dj][1]
                        k = max(1, min(ct - 1, round(ct * o_dve_frac)))
                        nc.vector.tensor_tensor(
                            _cut(o, 0, k, dj), _cut(qv, 0, k, dj),
                            _cut(tv, 0, k, dj), OP.subtract)
                        nc.gpsimd.tensor_tensor(
                            _cut(o, k, ct, dj), _cut(qv, k, ct, dj),
                            _cut(tv, k, ct, dj), OP.subtract)
                    u = c["u"]
                    if u is None:
                        continue
                    if fp16 and s <= 16:
                        clamp_band2_u(s, u)
                        if s < 16:
                            nc.scalar.activation(c["ue"], c["e"], AF.Sqrt)
                            nc.scalar.activation(c["uo"], c["o"], AF.Sqrt)
                    if finish is not None:
                        finish(u)

            # layout2 fp16 cancellation clamps, restricted to quarter u.
            CLAMP2 = {
                13: [(8, [[512, 8], [16, 8], [1, 8]]),
                     (384, [[512, 8], [16, 8], [1, 8]])],
                14: [(16, [[512, 8], [32, 4], [1, 8]]),
                     (392, [[512, 8], [32, 4], [1, 8]])],
                15: [(32, [[512, 8], [64, 4], [1, 8]]),
                     (280, [[512, 8], [64, 4], [1, 8]])],
                16: [(64, [[128, 32], [1, 8]]),
                     (56, [[128, 32], [1, 8]])],
            }

            def clamp_band2_u(s, u):
                for off, dims in CLAMP2[s]:
                    # dims whose span stays inside one 128-col block keep
                    # their AP form; the rest are enumerated so cols can be
                    # quarter-filtered (u = col bits 7..8).
                    hi = [dx for dx in dims
                          if dx[0] >= 128 or dx[0] * dx[1] > 128]
                    lo = [dx for dx in dims if dx not in hi]
                    combos = [0]
                    for stx, ctx in hi:
                        combos = [c0 + stx * i for c0 in combos
                                  for i in range(ctx)]
                    cols = sorted(c0 + off for c0 in combos
                                  if ((c0 + off) >> 7) & 3 == u)
                    groups = {}
                    for c0 in cols:
                        groups.setdefault(c0 % 512, []).append(c0)
                    for gcols in groups.values():
                        if len(gcols) > 1:
                            stq = gcols[1] - gcols[0]
                            assert all(gcols[i + 1] - gcols[i] == stq
                                       for i in range(len(gcols) - 1)), \
                                (s, gcols)
                            bdims = [[stq, len(gcols)]] + lo
                        else:
                            bdims = lo if lo else [[1, 1]]
                        band = _ap(v2[:], gcols[0], bdims)
                        nc.vector.tensor_scalar(band, band, 0.0, None,
                                                OP.max)

            # ------------- per-quarter pipeline ---------------------------
            # transpose + send + a2a + recv for one quarter (chained onto
            # the stage-12 epilogue so quarter u ships while u+1 computes)
            def ship(u):
                stg = sp.tile([P, 1024], FP16, tag="stg")
                for bi in range(0, M, 4):
                    pt = pp.tile([P, 512], FP16, tag="pt")
                    for j, dp in enumerate(range(bi, bi + 4)):
                        c0 = dp * 512 + u * 128
                        nc.tensor.transpose(pt[:, j * 128:(j + 1) * 128],
                                            v16[:, c0:c0 + 128], ident_t[:])
                    dst = stg[:, bi * 128:(bi + 4) * 128]
                    if bi == 0:
                        nc.vector.tensor_scalar(dst, pt[:], 1.0, None,
                                                OP.mult)
                    else:
                        nc.scalar.copy(dst, pt[:])
                nc.sync.dma_start(
                    sendh[u][:].rearrange("d w c -> w d c"),
                    stg[:].rearrange("w (d c) -> w d c", d=M))
                if no_cc:
                    # transport stand-in; ACT's next compute (stage-13
                    # sqrts) waits on the recvs regardless
                    nc.scalar.dma_start(recvh[u][:], sendh[u][:])
                else:
                    nc.gpsimd.collective_compute(
                        "AllToAll", OP.bypass,
                        replica_groups=[list(range(M))],
                        ins=[sendh[u][:].opt()],
                        outs=[recvh[u][:].opt()])
                if u == 3:
                    # recvs go last on SP so they never head-block sends
                    # (a recv waiting on its collective would stall any
                    # queue it shares with later compute or sends).
                    for ur in range(4):
                        nc.sync.dma_start(
                            v2[:].rearrange("w (s u c) -> w s u c",
                                            s=M, u=4)[:, :, ur, :],
                            recvh[ur][:].rearrange("s w c -> w s c"))
                    # layout2 sqrt-state seed, one op per quarter (the ACT
                    # engine is otherwise idle during the a2a)
                    for ur in range(4):
                        nc.scalar.activation(
                            _ap(u2[:], ur * 128, [[512, 8], [1, 128]]),
                            _ap(v2[:], ur * 128, [[512, 8], [1, 128]]),
                            AF.Sqrt)

            # final magnitudes + output DMA for one quarter (two halves so
            # the last quarter's tail is short)
            def final(u):
                ot = sp.tile([P, 1024], FP16, tag="ot")
                ov = out[:].rearrange("w (s u c) -> w s u c",
                                      s=M, u=4)[:, :, u, :]
                otv = ot[:].rearrange("w (s c) -> w s c", s=M)
                for (a, bnd) in ((0, 4), (4, 8)):
                    nc.scalar.activation(
                        _ap(ot[:], a * 128, [[128, bnd - a], [1, 128]]),
                        _ap(v32[:], u * 128 + a * 512,
                            [[512, bnd - a], [1, 128]]),
                        AF.Sqrt, scale=FINAL_SCALE * OUT_SCALE * OUT_SCALE)
                    eng = nc.sync if (u + a) % 2 == 0 else nc.scalar
                    eng.dma_start(ov[:, a:bnd, :], otv[:, a:bnd, :])

            if not stopped:
                stage_l1_allu(10, 10 in RESCALE_AFTER)
                stage_l1_allu(11, False)
                stage_l1_allu(12, 12 in RESCALE_AFTER, finish=ship)
                stopped = bail('a2a', v2[:])
            for s in range(13, 17):
                if stopped:
                    break
                stage_l2_allu(s, True, 1.0)
                stopped = stopped or bail(f'stage{s}', v2[:])
            if not stopped:
                stage_l2_allu(17, False, 0.5, in16=True)
                for s in range(18, 22):
                    stage_l2_allu(s, False, O32)
                stage_l2_allu(22, False, O32, finish=final)

    nc.finalize()
    return nc


_NC_CACHE = None


def _get_nc():
    global _NC_CACHE
    if _NC_CACHE is None:
        _NC_CACHE = build_nc()
    return _NC_CACHE


def host_inputs(x):
    perm = _in_perm()
    xv = x[perm].astype(np.float16)          # [M, P, F] fp16
    ident = np.eye(P, dtype=np.float16)
    twe = host_tw_early()
    return [dict(x=xv[d], twe=twe, twl16=host_tw_late16(d),
                 twl32=host_tw_late32(d), ident=ident) for d in range(M)]


def assemble(outs):
    operm = _out_perm()
    full = np.empty(N, dtype=np.float32)
    inv = np.float32(1.0 / OUT_SCALE)
    for d in range(M):
        full[operm[d].reshape(-1)] = (
            np.asarray(outs[d]).astype(np.float32).reshape(-1) * inv)
    return full


def kernel(x: np.ndarray) -> np.ndarray:
    x = np.asarray(x)
    assert x.shape == (N,) and x.dtype == np.float32, (x.shape, x.dtype)
    in_maps = host_inputs(x)
    nc = _get_nc()
    res = run_bass_kernel_spmd(nc, in_maps, core_ids=list(range(M)))
    return assemble([res.results[d]["out"] for d in range(M)])


if __name__ == "__main__":
    rng = np.random.default_rng(0)
    x = rng.standard_normal(N).astype(np.float32)
    r = kernel(x)
    print("kernel ran, out[:4] =", r[:4])


# revision 59
# speedup vs baseline: 1.1304x; 1.0210x over previous
"""Distributed 2^22-point radix-2 FFT-with-abs-at-every-stage on 8 NeuronCores.

Math: the reference applies abs() after every butterfly stage, so all state is
real and non-negative.  We propagate SQUARED magnitudes v = |y|^2:
    stage s:  q = v_e + v_o ;  r = sqrt(v_e * v_o)
              v_e' = q + 2*cos(2*pi*k/2^s)*r ;  v_o' = q - 2*c*r
(no sines needed: |e + (c - i s) o|^2 = e^2 + o^2 + 2 c e o for e,o >= 0).

Precision: stages 1..16 fp16 (DVE 2x_1p packing), 17..22 fp32.  Flat x2^-4
rescales after stages 6,8,10,12,14 keep fp16 in range; the product (2^-20)
is undone in the final sqrt's free ACT scale.  Rescales fold into the stage
(q scaled by one TS op, r scaled inside the ACT sqrt) - never a separate
pass.  Input is cast fp16 on host; output is fp16 scaled by 2^-7 and upcast
on host (power-of-2 scales are error-free).

Distribution (one all-to-all; butterflies pair along the free dimension):
  - host bit-reverses x and shards contiguously: core d owns bits 21..19 = d
  - layout1 [128, 4096]: partition w = permuted bits 18..12
    (w = [b16 b17 b18 | b12 b13 b14 b15], LSB first), free f = bits 11..0
    -> stages 1..12 pair free bits 0..11
  - PE-transpose + AllToAll resharding -> layout2 [128, 4096]:
    partition ww = bits 6..0, f' = [b16 b17 b18 | b12..b15 | b7 b8 | b19..b21]
    -> stages 13..16 pair f' bits 3..6 (fp16), 17..19 pair f' 0..2 (fp32),
       20..22 pair f' 9..11 (fp32).
Pipelining: f bits 7..8 (= f' bits 7..8 = the a2a quarter index u) are last
paired at stage 9, so stages 10..22 + transpose + a2a + final output all run
per-quarter: a 4-deep pipeline that hides the collective and output DMA.
Engines: DVE: q, t2, e', most of o'; Pool: p = v_e*v_o (fp32 out), rest of
o'; ACT: r = sqrt(p) (+ folded rescale), PSUM drains, final sqrt.
"""

import numpy as np

import concourse.bacc as bacc
import concourse.mybir as mybir
import concourse.tile as tile
from concourse.bass_types import AP
from concourse.bass_utils import run_bass_kernel_spmd

FP32 = mybir.dt.float32
FP16 = mybir.dt.float16
AF = mybir.ActivationFunctionType
OP = mybir.AluOpType

NBITS = 22
N = 1 << NBITS
M = 8          # cores
P = 128
F = 4096

RESCALE_AFTER = (6, 8, 10, 12, 14, 18)
RS = 2.0 ** -4
FINAL_SCALE = float(2.0 ** (4 * len(RESCALE_AFTER)))  # undo in final sqrt
OUT_SCALE = 2.0 ** -7         # fp16 output headroom; undone on host
SHRINK16 = 1.0
SHRINK32 = 1.0 - 2.0 ** -14

# f'-bit (layout2 free bit) paired by stage s
F_BIT = {}
for _s in range(13, 17):
    F_BIT[_s] = 3 + (_s - 13)      # global bits 12..15
for _s in range(17, 20):
    F_BIT[_s] = _s - 17            # global bits 16..18
for _s in range(20, 23):
    F_BIT[_s] = 9 + (_s - 20)      # global bits 19..21

TWL16_STAGES = (13, 14, 15, 16, 18, 19)                  # fp16 late stages
TWL32_STAGES = (17, 20, 21, 22)                          # fp32-path stages
TWL_OFF = {s: 2048 * i for i, s in enumerate(TWL16_STAGES)}
TWL32_OFF = {s: 2048 * i for i, s in enumerate(TWL32_STAGES)}

TWE_OFF = {}
_c = 0
for _s in range(2, 13):
    TWE_OFF[_s] = _c
    _c += 1 << (_s - 1)
TWE_COLS = _c


def _bitrev_perm():
    x = np.arange(N)
    r = np.zeros(N, dtype=np.int64)
    for b in range(NBITS):
        r = (r << 1) | ((x >> b) & 1)
    return r


def _fprime_to_n(fp, ww, d):
    """global index n from layout2 (core d, partition ww, free col f')."""
    return (ww
            + ((fp >> 0) & 7) * (1 << 16)
            + ((fp >> 3) & 15) * (1 << 12)
            + ((fp >> 7) & 3) * (1 << 7)
            + d * (1 << 9)
            + ((fp >> 9) & 7) * (1 << 19))


# --------------------------------------------------------------------------
# host-side tables / permutations (cached)
# --------------------------------------------------------------------------
_HOST_CACHE = {}


def _in_perm():
    """perm[d, w, f] -> index into x (full input)."""
    if "in" not in _HOST_CACHE:
        br = _bitrev_perm()
        w = np.arange(P)[:, None]
        f = np.arange(F)[None, :]
        l = f + ((w >> 3) & 15) * (1 << 12) + (w & 7) * (1 << 16)
        out = np.empty((M, P, F), dtype=np.int64)
        for d in range(M):
            out[d] = br[d * (1 << 19) + l]
        _HOST_CACHE["in"] = out
    return _HOST_CACHE["in"]


def _out_perm():
    """perm[d, ww, f'] -> global output index n."""
    if "out" not in _HOST_CACHE:
        ww = np.arange(P)[:, None]
        fp = np.arange(F)[None, :]
        out = np.empty((M, P, F), dtype=np.int64)
        for d in range(M):
            out[d] = _fprime_to_n(fp, ww, d)
        _HOST_CACHE["out"] = out
    return _HOST_CACHE["out"]


def host_tw_early():
    """[P, TWE_COLS] fp16: stages 2..12, cols 2^(s-1) each, identical rows.
    Rescale-stage slices are pre-scaled by RS (the q path applies RS with
    one TS op; t2 = tw*r picks it up from the table)."""
    if "twE" in _HOST_CACHE:
        return _HOST_CACHE["twE"]
    cols = []
    for s in range(2, 13):
        h = 1 << (s - 1)
        t = np.arange(h)
        rsc = RS if s in RESCALE_AFTER else 1.0
        cols.append(2.0 * np.cos(2 * np.pi * t / (1 << s)) * SHRINK16 * rsc)
    row = np.concatenate(cols).astype(np.float16)
    _HOST_CACHE["twE"] = np.broadcast_to(row, (P, row.size)).copy()
    return _HOST_CACHE["twE"]


def _late_table(s, d, dtype, shrink):
    """[P, 2048] twiddle table for stage s, core d: col idx = f' with
    pairing bit F_BIT[s] removed; value 2*cos(2*pi*k/2^s)*shrink."""
    b = F_BIT[s]
    idx = np.arange(2048)
    fp = (idx & ((1 << b) - 1)) | ((idx >> b) << (b + 1))
    ww = np.arange(P)[:, None]
    n = _fprime_to_n(fp[None, :], ww, d)
    k = n % (1 << (s - 1))
    return (2.0 * np.cos(2 * np.pi * k / (1 << s)) * shrink).astype(dtype)


def host_tw_late16(d):
    key = ("twL16", d)
    if key not in _HOST_CACHE:
        _HOST_CACHE[key] = np.concatenate(
            [_late_table(s, d, np.float16, SHRINK16) *
             np.float16(RS if s in RESCALE_AFTER else 1.0)
             for s in TWL16_STAGES], axis=1)
    return _HOST_CACHE[key]


def host_tw_late32(d):
    key = ("twL32", d)
    if key not in _HOST_CACHE:
        _HOST_CACHE[key] = np.concatenate(
            [_late_table(s, d, np.float32, SHRINK32) for s in TWL32_STAGES],
            axis=1)
    return _HOST_CACHE[key]


def _clamp_runs():
    """Cancellation-clamp column runs for the fp16 stages 18/19: for each
    stage, a list of (u, col0, stride, count) stride-512-x-8 run specs
    (cols col0 + stride*i + 512*j).  Union over (ww, d) - over-clamping
    positive values is a no-op."""
    if "cr" in _HOST_CACHE:
        return _HOST_CACHE["cr"]
    thr = 2.0 * np.cos(np.pi / 16)
    out = {}
    for s in (18, 19):
        b = F_BIT[s]
        fp = np.arange(4096)
        ww = np.arange(P)[:, None, None]
        d = np.arange(M)[None, :, None]
        n = _fprime_to_n(fp[None, None, :], ww, d)
        k = n % (1 << (s - 1))
        c2 = 2.0 * np.cos(2 * np.pi * k / (1 << s))
        eside = ((fp >> b) & 1) == 0
        clamp = np.where(eside, c2.min(axis=(0, 1)) <= -thr,
                         c2.max(axis=(0, 1)) >= thr)
        res = sorted(set(int(c) % 512 for c in np.nonzero(clamp)[0]))
        runs = []
        i = 0
        while i < len(res):
            j = i
            stq = res[i + 1] - res[i] if i + 1 < len(res) else 1
            while (j + 1 < len(res) and res[j + 1] - res[j] == stq
                   and (res[j + 1] >> 7) == (res[i] >> 7)):
                j += 1
            runs.append(((res[i] >> 7) & 3, res[i], stq, j - i + 1))
            i = j + 1
        out[s] = runs
    _HOST_CACHE["cr"] = out
    return out


# --------------------------------------------------------------------------
# AP helpers
# --------------------------------------------------------------------------
def _ap(base, col_off, dims):
    """AP into a [P, C] tile at column offset with explicit free dims."""
    sl = base[:, col_off:] if col_off else base
    return AP(sl.tensor, sl.offset, [sl.ap[0]] + dims)


def _halve_strides(dims, b):
    """table AP dims from data AP dims when the table has pairing bit b
    removed: strides > 2^b halve; equal/lower stay."""
    thr = 1 << b
    return [[(st // 2 if st > thr else st), ct] for st, ct in dims]


def _compact(dims):
    """scratch-tile dims with the same counts, row-major compacted."""
    out = []
    acc = 1
    for st, ct in reversed(dims):
        out.insert(0, [acc, ct])
        acc *= ct
    return out


def _l2_dims(s):
    """(e-side free dims, e->o column delta) for a layout2 quarter: cols
    sblk*512 + u*128 + c relative to the u*128 base; stage s pairs f' bit
    b=F_BIT[s] (inside c for b<=6, inside sblk for b>=9)."""
    b = F_BIT[s]
    if b <= 6:
        lo = 1 << b
        hi = 64 // lo
        dims = [[512, 8]]
        if hi > 1:
            dims.append([2 * lo, hi])
        if lo > 1:
            dims.append([1, lo])
        return dims, lo
    j = b - 9
    lo = 1 << j
    hi = 4 // lo
    dims = []
    if hi > 1:
        dims.append([2 * lo * 512, hi])
    if lo > 1:
        dims.append([512, lo])
    dims.append([1, 128])
    return dims, lo * 512


def _cut(ap_, lo, hi, dim_idx):
    """restrict free dim dim_idx (0-based among free dims) to [lo, hi)."""
    dd = [list(x) for x in ap_.ap]
    st = dd[1 + dim_idx][0]
    dd[1 + dim_idx][1] = hi - lo
    return AP(ap_.tensor, ap_.offset + st * lo, dd)


# --------------------------------------------------------------------------
# device program
# --------------------------------------------------------------------------
def build_nc(stop_after=None, no_cc=False, NCH=4, O16=1.0, O32=0.27):
    nc = bacc.Bacc()

    x_in = nc.dram_tensor("x", [P, F], FP16, kind="ExternalInput")
    twe_in = nc.dram_tensor("twe", [P, TWE_COLS], FP16, kind="ExternalInput")
    twl16_in = nc.dram_tensor("twl16", [P, 12288], FP16,
                              kind="ExternalInput")
    twl32_in = nc.dram_tensor("twl32", [P, 8192], FP32, kind="ExternalInput")
    ident_in = nc.dram_tensor("ident", [P, P], FP16, kind="ExternalInput")
    out = nc.dram_tensor("out", [P, F], FP16, kind="ExternalOutput")

    with tile.TileContext(nc) as tc:
        _handles = []
        sendh = []
        recvh = []
        for uh in range(4):
            s_t, _h1 = tc.tile([M, P, 128], FP16, space="DRAM",
                               name=f"a2a_send{uh}")
            r_t, _h2 = tc.tile([M, P, 128], FP16, space="DRAM",
                               addr_space="Shared", name=f"a2a_recv{uh}")
            _handles += [_h1, _h2]
            sendh.append(s_t)
            recvh.append(r_t)

        v16, _h3 = tc.tile([P, F], FP16, name="v16")     # layout1 state
        v2, _h4 = tc.tile([P, F], FP16, name="v2")       # layout2 fp16 state
        v32, _h5 = tc.tile([P, F], FP32, name="v32")     # layout2 fp32 state
        u16, _hu1 = tc.tile([P, F], FP16, name="u16")    # sqrt(v16) state
        u2, _hu2 = tc.tile([P, F], FP16, name="u2")      # sqrt(v2) state
        twe, _h6 = tc.tile([P, TWE_COLS], FP16, name="twe")
        twl16, _h7 = tc.tile([P, 12288], FP16, name="twl16")
        twl32, _h8 = tc.tile([P, 8192], FP32, name="twl32")
        ident_t, _h9 = tc.tile([P, P], FP16, name="ident")

        with (
            tc.tile_pool(name="io", bufs=1) as io_pool,
            tc.tile_pool(name="scratch", bufs=4) as sp,
            tc.tile_pool(name="psum", bufs=4, space="PSUM") as pp,
        ):
            xt = io_pool.tile([P, F], FP16, tag="io")
            x_engs = (nc.sync, nc.scalar, nc.gpsimd, nc.sync)
            X_CH = (0, 1024, 2048, 3072, 4096)
            for ci in range(4):
                x_engs[ci].dma_start(xt[:, X_CH[ci]:X_CH[ci + 1]],
                                     x_in[:, X_CH[ci]:X_CH[ci + 1]])
            # twe needed by stage 2 (~5us): cheap dispatch on the Pool queue.
            nc.gpsimd.dma_start(twe[:], twe_in[:])
            # The rest rides the otherwise-idle SP queue behind x.
            nc.sync.dma_start(ident_t[:], ident_in[:])
            for ci in range(3):
                nc.sync.dma_start(twl16[:, ci * 4096:(ci + 1) * 4096],
                                  twl16_in[:, ci * 4096:(ci + 1) * 4096])
            for ci in range(2):
                nc.sync.dma_start(twl32[:, ci * 4096:(ci + 1) * 4096],
                                  twl32_in[:, ci * 4096:(ci + 1) * 4096])

            def bail(label, ap, scale=1.0):
                """debug: dump fp16(scale*ap) and stop emitting stages."""
                if stop_after == label:
                    ot = io_pool.tile([P, F], FP16, tag="bailio")
                    nc.scalar.mul(ot[:], ap, scale)
                    nc.sync.dma_start(out[:], ot[:])
                    return True
                return False

            # ---------------- stage 1 (fp16 in -> fp16 squares) -----------
            xe = xt[:].rearrange("p (c pair) -> p c pair", pair=2)[:, :, 0]
            xo = xt[:].rearrange("p (c pair) -> p c pair", pair=2)[:, :, 1]
            ve = v16[:].rearrange("p (c pair) -> p c pair", pair=2)[:, :, 0]
            vo = v16[:].rearrange("p (c pair) -> p c pair", pair=2)[:, :, 1]
            ue = u16[:].rearrange("p (c pair) -> p c pair", pair=2)[:, :, 0]
            uo = u16[:].rearrange("p (c pair) -> p c pair", pair=2)[:, :, 1]
            s1 = []
            for ci in range(4):
                sl = slice(ci * 512, (ci + 1) * 512)
                st = sp.tile([P, 512], FP16, tag="s1s")
                dt = sp.tile([P, 512], FP16, tag="s1d")
                nc.vector.tensor_tensor(st[:], xe[:, sl], xo[:, sl], OP.add)
                nc.gpsimd.tensor_tensor(dt[:], xe[:, sl], xo[:, sl],
                                        OP.subtract)
                s1.append((sl, st, dt))
            for (sl, st, dt) in s1:
                nc.vector.tensor_tensor(ve[:, sl], st[:], st[:], OP.mult)
                nc.vector.tensor_tensor(vo[:, sl], dt[:], dt[:], OP.mult)
                nc.scalar.activation(ue[:, sl], ve[:, sl], AF.Sqrt)
                nc.scalar.activation(uo[:, sl], vo[:, sl], AF.Sqrt)
            stopped = bail('stage1', v16[:])

            # ---------------- full-width stages 2..9 ----------------------
            # u-state form: r = u_e*u_o on DVE (no Pool product / ACT sqrt
            # on the critical path); ACT computes the two OUTPUT sqrts that
            # feed the next stage.  Chunk fractions: small lead chunk so
            # the pipeline primes, small tail chunk so the next stage's
            # first sqrt lands early.
            def do_stage(s, rescale, nch):
                b = s - 1
                vv = v16[:].rearrange("p (o pair i) -> p o pair i",
                                      pair=2, i=1 << b)
                uu = u16[:].rearrange("p (o pair i) -> p o pair i",
                                      pair=2, i=1 << b)
                e_all, o_all = vv[:, :, 0, :], vv[:, :, 1, :]
                ue_all, uo_all = uu[:, :, 0, :], uu[:, :, 1, :]
                n_out = 1 << (11 - b)
                inner = 1 << b
                nblk = n_out // 4
                cd = []
                for ci in range(4):
                    co = slice(ci * nblk, (ci + 1) * nblk)
                    ccols = nblk * inner
                    tw = _ap(twe[:], TWE_OFF[s], [[0, nblk], [1, inner]])
                    q = sp.tile([P, ccols], FP16, tag="q")
                    r2 = sp.tile([P, ccols], FP16, tag="r")
                    t2 = sp.tile([P, ccols], FP16, tag="t")
                    mk = lambda t_: t_.rearrange("p (o i) -> p o i", i=inner)
                    cd.append(dict(e=e_all[:, co, :], o=o_all[:, co, :],
                                   uei=ue_all[:, co, :],
                                   uoi=uo_all[:, co, :],
                                   tw=tw, q=q, r2=r2, t2=t2,
                                   qv=mk(q[:]), rv=mk(r2[:]), tv=mk(t2[:])))
                rsc = RS if rescale else 1.0
                # q on Pool (input-only dep: starts at stage begin);
                # r/t2/e'/o' on DVE; clamps per chunk; sqrts per half.
                for c in cd:
                    nc.gpsimd.tensor_tensor(c["qv"], c["e"], c["o"], OP.add)
                    nc.vector.tensor_tensor(c["rv"], c["uei"], c["uoi"],
                                            OP.mult)
                for ci, c in enumerate(cd):
                    if rescale:
                        nc.vector.tensor_scalar(c["q"][:], c["q"][:], rsc,
                                                None, OP.mult)
                    nc.vector.tensor_tensor(c["tv"], c["tw"], c["rv"],
                                            OP.mult)
                    nc.vector.tensor_tensor(c["e"], c["qv"], c["tv"], OP.add)
                    nc.vector.tensor_tensor(c["o"], c["qv"], c["tv"],
                                            OP.subtract)
                    clamp_band(s, ci * nblk, nblk)
                    if ci % 2 == 1:
                        a, bnd = (ci - 1) * nblk, (ci + 1) * nblk
                        nc.scalar.activation(ue_all[:, a:bnd, :],
                                             e_all[:, a:bnd, :], AF.Sqrt)
                        nc.scalar.activation(uo_all[:, a:bnd, :],
                                             o_all[:, a:bnd, :], AF.Sqrt)

            def clamp_band(s, blk0, nblk):
                """max(v,0) on the cancellation band, outer blocks
                [blk0, blk0+nblk) only."""
                h = 1 << (s - 1)
                m = 2 * h
                g = max(1, h // 16)
                base = v16[:, blk0 * m + h - g:]
                band = AP(base.tensor, base.offset,
                          [base.ap[0], [m, nblk], [1, 2 * g]])
                nc.vector.tensor_scalar(band, band, 0.0, None, OP.max)

            for s in range(2, 10):
                if stopped:
                    break
                do_stage(s, s in RESCALE_AFTER, NCH)
                stopped = stopped or bail(f'stage{s}', v16[:])

            # ------------- per-quarter stages 10..12 (layout1) ------------
            # phase-major across the 4 quarters (u = the chunk index)
            def stage_l1_allu(s, rescale, finish=None, us=(0, 1, 2, 3)):
                j = s - 10                      # dp bit paired
                lo = 1 << j
                hi = 4 // lo
                dims = []
                tdims = []
                sdims = []
                if hi > 1:
                    dims.append([2 * lo * 512, hi])
                    tdims.append([0, hi])
                    sdims.append([lo * 128, hi])
                if lo > 1:
                    dims.append([512, lo])
                    tdims.append([512, lo])
                    sdims.append([128, lo])
                dims.append([1, 128])
                tdims.append([1, 128])
                sdims.append([1, 128])
                rsc = RS if rescale else 1.0
                last = s == 12
                cd = []
                for u in us:
                    q = sp.tile([P, 512], FP16, tag="q")
                    r2 = sp.tile([P, 512], FP16, tag="r")
                    t2 = sp.tile([P, 512], FP16, tag="t")
                    cd.append(dict(
                        u=u,
                        e=_ap(v16[:], u * 128, dims),
                        o=_ap(v16[:], u * 128 + lo * 512, dims),
                        ue=_ap(u16[:], u * 128, dims),
                        uo=_ap(u16[:], u * 128 + lo * 512, dims),
                        tw=_ap(twe[:], TWE_OFF[s] + u * 128, tdims),
                        qv=_ap(q[:], 0, sdims), q=q,
                        rv=_ap(r2[:], 0, sdims), tv=_ap(t2[:], 0, sdims)))
                for c in cd:
                    nc.gpsimd.tensor_tensor(c["qv"], c["e"], c["o"], OP.add)
                    nc.vector.tensor_tensor(c["rv"], c["ue"], c["uo"],
                                            OP.mult)
                for c in cd:
                    if rescale:
                        nc.vector.tensor_scalar(c["q"][:], c["q"][:], rsc,
                                                None, OP.mult)
                    nc.vector.tensor_tensor(c["tv"], c["tw"], c["rv"],
                                            OP.mult)
                    nc.vector.tensor_tensor(c["e"], c["qv"], c["tv"], OP.add)
                    nc.vector.tensor_tensor(c["o"], c["qv"], c["tv"],
                                            OP.subtract)
                    clamp_band_l1u(s, c["u"])
                    if not last:
                        nc.scalar.activation(c["ue"], c["e"], AF.Sqrt)
                        nc.scalar.activation(c["uo"], c["o"], AF.Sqrt)
                    if finish is not None:
                        finish(c["u"])

            def clamp_band_l1u(s, u):
                """layout1 clamp band (col mod 2h in [h-g, h+g)) restricted
                to quarter u, at 64-col granularity (over-clamp is safe)."""
                h = 1 << (s - 1)
                g = h // 16
                cols = []
                for dp in range(8):
                    base = dp * 512 + u * 128
                    for cb in (base, base + 64):
                        kk = cb % (2 * h)
                        if kk + 64 > h - g and kk < h + g:
                            cols.append(cb)
                i = 0
                while i < len(cols):
                    jf = i
                    while jf + 1 < len(cols) and cols[jf + 1] == cols[jf] + 64:
                        jf += 1
                    band = v16[:, cols[i]:cols[jf] + 64]
                    nc.vector.tensor_scalar(band, band, 0.0, None, OP.max)
                    i = jf + 1

            # ------------- layout2 stage helper (phase-major over u) ------
            def stage_l2_allu(s, fp16, o_dve_frac, in16=None, finish=None,
                              df=False, out16=False):
                b = F_BIT[s]
                dims, d_pair = _l2_dims(s)
                dt_state = FP16 if fp16 else FP32
                vt = v2 if (fp16 or out16) else v32
                vin = v2 if (fp16 or in16) else v32
                tw_t = twl16 if fp16 else twl32
                t0 = (TWL_OFF if fp16 else TWL32_OFF)[s]
                sdims = _compact(dims)
                di = max(range(len(dims)), key=lambda i: dims[i][1])
                dct = dims[di][1]
                cd = []
                for u in range(4):
                    q = sp.tile([P, 512], dt_state, tag="q")
                    p32 = sp.tile([P, 512], FP32, tag="p")
                    r2 = sp.tile([P, 512], dt_state, tag="r")
                    t2 = sp.tile([P, 512], dt_state, tag="t")
                    toff = u * (64 if b < 7 else 128)
                    full = dict(
                        ein=_ap(vin[:], u * 128, dims),
                        oin=_ap(vin[:], u * 128 + d_pair, dims),
                        e=_ap(vt[:], u * 128, dims),
                        o=_ap(vt[:], u * 128 + d_pair, dims),
                        ue=_ap(u2[:], u * 128, dims),
                        uo=_ap(u2[:], u * 128 + d_pair, dims),
                        tw=_ap(tw_t[:], t0 + toff, _halve_strides(dims, b)),
                        qv=_ap(q[:], 0, sdims), pv=_ap(p32[:], 0, sdims),
                        rv=_ap(r2[:], 0, sdims), tv=_ap(t2[:], 0, sdims))
                    if u == 0 and not fp16:
                        h = dct // 2
                        for (a, bnd) in ((0, h), (h, dct)):
                            cc = {k: _cut(v, a, bnd, di)
                                  for k, v in full.items()}
                            cc["u"] = u if bnd == dct else None
                            cd.append(cc)
                    else:
                        full["u"] = u
                        cd.append(full)
                def phase1(c):
                    if fp16:
                        nc.gpsimd.tensor_tensor(c["qv"], c["ein"],
                                                c["oin"], OP.add)
                        nc.vector.tensor_tensor(c["rv"], c["ue"], c["uo"],
                                                OP.mult)
                    else:
                        nc.vector.tensor_tensor(c["qv"], c["ein"],
                                                c["oin"], OP.add)
                        nc.gpsimd.tensor_tensor(c["pv"], c["ein"],
                                                c["oin"], OP.mult)
                if not df:
                    for c in cd:
                        phase1(c)
                    if not fp16:
                        for c in cd:
                            nc.scalar.activation(c["rv"], c["pv"], AF.Sqrt)
                for c in cd:
                    if df:
                        # depth-first per quarter: chains follow each
                        # recv's arrival instead of waiting for the last
                        phase1(c)
                        if not fp16:
                            nc.scalar.activation(c["rv"], c["pv"], AF.Sqrt)
                    o, qv, tv = c["o"], c["qv"], c["tv"]
                    if fp16 and s in RESCALE_AFTER:
                        nc.vector.tensor_scalar(qv, qv, RS, None, OP.mult)
                    nc.vector.tensor_tensor(tv, c["tw"], c["rv"], OP.mult)
                    nc.vector.tensor_tensor(c["e"], qv, tv, OP.add)
                    cdims = [list(x) for x in o.ap[1:]]
                    if fp16 or o_dve_frac >= 1.0:
                        nc.vector.tensor_tensor(o, qv, tv, OP.subtract)
                    else:
                        dj = max(range(len(cdims)),
                                 key=lambda i: cdims[i][1])
                        ct = cdims[dj][1]
                        k = max(1, min(ct - 1, round(ct * o_dve_frac)))
                        nc.vector.tensor_tensor(
                            _cut(o, 0, k, dj), _cut(qv, 0, k, dj),
                            _cut(tv, 0, k, dj), OP.subtract)
                        nc.gpsimd.tensor_tensor(
                            _cut(o, k, ct, dj), _cut(qv, k, ct, dj),
                            _cut(tv, k, ct, dj), OP.subtract)
                    u = c["u"]
                    if fp16 and u is not None:
                        clamp_band2_u(s, u)
                    if s in (13, 14, 15, 18) and u is not None or s == 17:
                        # sqrt-state for the next fp16-u stage, emitted per
                        # chunk so stage 17's split halves are all covered
                        # (17's fp32 compute + shrink keeps v2 >= 0 there)
                        nc.scalar.activation(c["ue"], c["e"], AF.Sqrt)
                        nc.scalar.activation(c["uo"], c["o"], AF.Sqrt)
                    if u is not None and finish is not None:
                        finish(u)

            # layout2 fp16 cancellation clamps, restricted to quarter u.
            CLAMP2 = {
                13: [(8, [[512, 8], [16, 8], [1, 8]]),
                     (384, [[512, 8], [16, 8], [1, 8]])],
                14: [(16, [[512, 8], [32, 4], [1, 8]]),
                     (392, [[512, 8], [32, 4], [1, 8]])],
                15: [(32, [[512, 8], [64, 4], [1, 8]]),
                     (280, [[512, 8], [64, 4], [1, 8]])],
                16: [(64, [[128, 32], [1, 8]]),
                     (56, [[128, 32], [1, 8]])],
            }

            def clamp_band2_u(s, u):
                if s in (18, 19):
                    for (ru, col0, stq, ct) in _clamp_runs()[s]:
                        if ru != u:
                            continue
                        bdims = [[512, 8]]
                        bdims.append([stq, ct] if ct > 1 else [1, 1])
                        band = _ap(v2[:], col0, bdims)
                        nc.vector.tensor_scalar(band, band, 0.0, None,
                                                OP.max)
                    return
                for off, dims in CLAMP2[s]:
                    # dims whose span stays inside one 128-col block keep
                    # their AP form; the rest are enumerated so cols can be
                    # quarter-filtered (u = col bits 7..8).
                    hi = [dx for dx in dims
                          if dx[0] >= 128 or dx[0] * dx[1] > 128]
                    lo = [dx for dx in dims if dx not in hi]
                    combos = [0]
                    for stx, ctx in hi:
                        combos = [c0 + stx * i for c0 in combos
                                  for i in range(ctx)]
                    cols = sorted(c0 + off for c0 in combos
                                  if ((c0 + off) >> 7) & 3 == u)
                    groups = {}
                    for c0 in cols:
                        groups.setdefault(c0 % 512, []).append(c0)
                    for gcols in groups.values():
                        if len(gcols) > 1:
                            stq = gcols[1] - gcols[0]
                            assert all(gcols[i + 1] - gcols[i] == stq
                                       for i in range(len(gcols) - 1)), \
                                (s, gcols)
                            bdims = [[stq, len(gcols)]] + lo
                        else:
                            bdims = lo if lo else [[1, 1]]
                        band = _ap(v2[:], gcols[0], bdims)
                        nc.vector.tensor_scalar(band, band, 0.0, None,
                                                OP.max)

            # ------------- per-quarter pipeline ---------------------------
            # transpose + send + a2a + recv for one quarter (chained onto
            # the stage-12 epilogue so quarter u ships while u+1 computes)
            def ship(u):
                stg = sp.tile([P, 1024], FP16, tag="stg")
                for bi in range(0, M, 4):
                    pt = pp.tile([P, 512], FP16, tag="pt")
                    for j, dp in enumerate(range(bi, bi + 4)):
                        c0 = dp * 512 + u * 128
                        nc.tensor.transpose(pt[:, j * 128:(j + 1) * 128],
                                            v16[:, c0:c0 + 128], ident_t[:])
                    dst = stg[:, bi * 128:(bi + 4) * 128]
                    if bi == 0:
                        nc.vector.tensor_scalar(dst, pt[:], 1.0, None,
                                                OP.mult)
                    else:
                        nc.scalar.copy(dst, pt[:])
                nc.sync.dma_start(
                    sendh[u][:].rearrange("d w c -> w d c"),
                    stg[:].rearrange("w (d c) -> w d c", d=M))
                if no_cc:
                    # transport stand-in; ACT's next compute (stage-13
                    # sqrts) waits on the recvs regardless
                    nc.scalar.dma_start(recvh[u][:], sendh[u][:])
                else:
                    nc.gpsimd.collective_compute(
                        "AllToAll", OP.bypass,
                        replica_groups=[list(range(M))],
                        ins=[sendh[u][:].opt()],
                        outs=[recvh[u][:].opt()])
                if u == 3:
                    # recvs go last on SP so they never head-block sends
                    # (a recv waiting on its collective would stall any
                    # queue it shares with later compute or sends).
                    for ur in range(4):
                        nc.sync.dma_start(
                            v2[:].rearrange("w (s u c) -> w s u c",
                                            s=M, u=4)[:, :, ur, :],
                            recvh[ur][:].rearrange("s w c -> w s c"))
                    # layout2 sqrt-state seed (ACT idles during the a2a)
                    for ur in range(4):
                        nc.scalar.activation(
                            _ap(u2[:], ur * 128, [[512, 8], [1, 128]]),
                            _ap(v2[:], ur * 128, [[512, 8], [1, 128]]),
                            AF.Sqrt)

            # final magnitudes + output DMA for one quarter (two halves so
            # the last quarter's tail is short)
            def final(u):
                ot = sp.tile([P, 1024], FP16, tag="ot")
                ov = out[:].rearrange("w (s u c) -> w s u c",
                                      s=M, u=4)[:, :, u, :]
                otv = ot[:].rearrange("w (s c) -> w s c", s=M)
                for (a, bnd) in ((0, 4), (4, 8)):
                    nc.scalar.activation(
                        _ap(ot[:], a * 128, [[128, bnd - a], [1, 128]]),
                        _ap(v32[:], u * 128 + a * 512,
                            [[512, bnd - a], [1, 128]]),
                        AF.Sqrt, scale=FINAL_SCALE * OUT_SCALE * OUT_SCALE)
                    eng = nc.sync if (u + a) % 2 == 0 else nc.scalar
                    eng.dma_start(ov[:, a:bnd, :], otv[:, a:bnd, :])

            if not stopped:
                stage_l1_allu(10, 10 in RESCALE_AFTER)
                stage_l1_allu(11, False)
                stage_l1_allu(12, 12 in RESCALE_AFTER, finish=ship)
                stopped = bail('a2a', v2[:])
            for s in range(13, 17):
                if stopped:
                    break
                stage_l2_allu(s, True, 1.0, df=(s == 13))
                stopped = stopped or bail(f'stage{s}', v2[:])
            if not stopped:
                stage_l2_allu(17, False, 0.5, in16=True, out16=True)
                stage_l2_allu(18, True, 1.0)
                stage_l2_allu(19, True, 1.0)
                stage_l2_allu(20, False, O32, in16=True)
                stage_l2_allu(21, False, O32)
                stage_l2_allu(22, False, O32, finish=final)

    nc.finalize()
    return nc


_NC_CACHE = None


def _get_nc():
    global _NC_CACHE
    if _NC_CACHE is None:
        _NC_CACHE = build_nc()
    return _NC_CACHE


def host_inputs(x):
    perm = _in_perm()
    xv = x[perm].astype(np.float16)          # [M, P, F] fp16
    ident = np.eye(P, dtype=np.float16)
    twe = host_tw_early()
    return [dict(x=xv[d], twe=twe, twl16=host_tw_late16(d),
                 twl32=host_tw_late32(d), ident=ident) for d in range(M)]


def assemble(outs):
    operm = _out_perm()
    full = np.empty(N, dtype=np.float32)
    inv = np.float32(1.0 / OUT_SCALE)
    for d in range(M):
        full[operm[d].reshape(-1)] = (
            np.asarray(outs[d]).astype(np.float32).reshape(-1) * inv)
    return full


def kernel(x: np.ndarray) -> np.ndarray:
    x = np.asarray(x)
    assert x.shape == (N,) and x.dtype == np.float32, (x.shape, x.dtype)
    in_maps = host_inputs(x)
    nc = _get_nc()
    res = run_bass_kernel_spmd(nc, in_maps, core_ids=list(range(M)))
    return assemble([res.results[d]["out"] for d in range(M)])


if __name__ == "__main__":
    rng = np.random.default_rng(0)
    x = rng.standard_normal(N).astype(np.float32)
    r = kernel(x)
    print("kernel ran, out[:4] =", r[:4])
